# revision 1
# baseline (speedup 1.0000x reference)
"""VQ codebook-lookup kernel for Trainium2 (8 NeuronCores, data-parallel).

Computes: idx = argmax(x, axis=-1); out = W.T[idx]  (i.e. out[n] = W[:, idx[n]])
  x: [8192, 8192] f32, W: [1024, 8192] f32 -> out: [8192, 1024] f32

Sharding: x split along tokens into 8 shards of [1024, 8192]; W.T replicated.
Each core: per-row argmax via VectorE max/max_index, then a DMA row-gather
from the transposed codebook in HBM.
"""

import sys

import numpy as np

sys.path.insert(0, "/opt/trn_rl_repo")

import concourse.bass as bass  # noqa: E402
import concourse.tile as tile  # noqa: E402
from concourse import bacc, bass_utils, mybir  # noqa: E402

N_CORES = 8
N_TOKENS = 8192
QUANT_DIM = 8192
OUT_DIM = 1024
ROWS = N_TOKENS // N_CORES  # rows (tokens) per core
P = 128  # SBUF partitions
N_TILES = ROWS // P  # row-tiles per core

F32 = mybir.dt.float32
I16 = mybir.dt.int16
I32 = mybir.dt.int32
U32 = mybir.dt.uint32

CH = 256  # chunk size for the hierarchical argmax
NCH = QUANT_DIM // CH  # 32 chunks per row

VERSION = 7
BATCHES = [(0, 4), (4, 3), (7, 1)]  # (first tile, n tiles) tail groups
WQ = 0  # single SWDGE queue (DMASW sem lanes are queue-locked; multi-queue trips Tile)


def _emit_kernel(tc: tile.TileContext, y: "bass.AP", x: "bass.AP", wt: "bass.AP"):
    """Per-core program. x: [ROWS, QUANT_DIM], wt: [QUANT_DIM, OUT_DIM] (=W.T),
    y: [ROWS, OUT_DIM]."""
    nc = tc.nc
    with (
        tc.tile_pool(name="xp", bufs=3) as xp,
        tc.tile_pool(name="sm", bufs=2 * N_TILES) as sm,
        tc.tile_pool(name="ip", bufs=1) as ip,
        tc.tile_pool(name="op", bufs=1) as op,
        tc.tile_pool(name="dr", bufs=1, space="DRAM") as dr,
    ):
        # Per-row argmax, one [128, QUANT_DIM] tile at a time.
        idx_all = ip.tile([P, N_TILES], I16)  # [row-in-tile, tile] argmax
        for t in range(N_TILES):
            xt = xp.tile([P, QUANT_DIM], F32)
            nc.sync.dma_start(xt[:], x[t * P : (t + 1) * P, :])
            mx = sm.tile([P, 8], F32, tag="mx")
            nc.vector.max(mx[:], xt[:])
            ix = sm.tile([P, 8], U32, tag="ix")
            nc.vector.max_index(ix[:], mx[:], xt[:])
            # argmax = ix[:, 0]; value < 8192 so the low half-word holds it.
            nc.vector.tensor_copy(idx_all[:, t : t + 1], ix.bitcast(I16)[:, 0:1])

        # dma_gather wants indices int16, "wrapped": gather j reads the index
        # at partition j%16, slot j//16 (replicated across the 8 gpsimd cores'
        # 16-partition groups). Row j = t*128 + p with p = s1*16 + q, so the
        # value for (q, slot=t*8+s1) is idx_all[s1*16+q, t]. Partition-crossing
        # shuffle goes through a DRAM scratch roundtrip.
        scratch = dr.tile([P, N_TILES], I16)
        nc.sync.dma_start(scratch[:], idx_all[:])
        idxw = ip.tile([P, ROWS // 16], I16)
        src = scratch.rearrange("(s1 q) t -> q t s1", q=16)
        for r in range(P // 16):
            dst = idxw[16 * r : 16 * (r + 1), :].rearrange(
                "q (t s1) -> q t s1", s1=N_TILES
            )
            nc.sync.dma_start(dst, src)

        # Gather rows of W.T from HBM: wout[p, t, :] = wt[idx[t*128+p], :]
        wout = op.tile([P, N_TILES, OUT_DIM], F32)
        nc.gpsimd.dma_gather(
            wout[:],
            wt[:],
            idxw[:],
            num_idxs=ROWS,
            num_idxs_reg=ROWS,
            elem_size=OUT_DIM,
        )
        nc.sync.dma_start(y.rearrange("(t p) d -> p t d", p=P), wout[:])


def _wrap_1024(nc, idx_sbuf, scratch, idxw):
    """Turn idx_sbuf [128, 8] int16 (value for row t*128+p at [p, t]) into the
    dma_gather wrapped layout idxw [128, 64]: gather j reads partition j%16,
    slot j//16; replicated across the 8 gpsimd-core partition groups.
    Row j = t*128 + s1*16 + q -> idxw[q, t*8+s1] = idx_sbuf[s1*16+q, t]."""
    nc.sync.dma_start(scratch[:], idx_sbuf[:])
    src = scratch.rearrange("(s1 q) t -> q t s1", q=16)
    for r in range(P // 16):
        dst = idxw[16 * r : 16 * (r + 1), :].rearrange(
            "q (t s1) -> q t s1", s1=N_TILES
        )
        nc.sync.dma_start(dst, src)


def _emit_kernel_v2(tc: tile.TileContext, y: "bass.AP", x: "bass.AP", wt: "bass.AP"):
    """Hierarchical argmax: one full pass computes per-chunk maxes (CH=256),
    cheap top-8 picks the winning chunk, a small HBM gather re-reads only the
    winning 1KB chunk per row, and a second tiny max/max_index finds the
    offset within it. ~1.06 passes of DVE work instead of 2."""
    nc = tc.nc
    with (
        tc.tile_pool(name="xp", bufs=3) as xp,
        tc.tile_pool(name="mp", bufs=N_TILES) as mp,
        tc.tile_pool(name="sm", bufs=2 * N_TILES) as sm,
        tc.tile_pool(name="keep", bufs=1) as keep,
        tc.tile_pool(name="op", bufs=1) as op,
        tc.tile_pool(name="dr", bufs=1, space="DRAM") as dr,
    ):
        # per-partition offsets: p*32 (global chunk id) as f32
        iota32 = keep.tile([P, 1], F32)
        nc.gpsimd.iota(
            iota32[:],
            pattern=[[0, 1]],
            base=0,
            channel_multiplier=NCH,
            allow_small_or_imprecise_dtypes=True,
        )

        cr_all = keep.tile([P, N_TILES], F32)  # winning chunk within row
        cidx_all = keep.tile([P, N_TILES], I16)  # global chunk id for gather
        # Phase A: chunk maxes + winning chunk per row.
        for t in range(N_TILES):
            xt = xp.tile([P, QUANT_DIM], F32)
            nc.sync.dma_start(xt[:], x[t * P : (t + 1) * P, :])
            m = mp.tile([P, NCH], F32, tag="m")
            nc.vector.reduce_max(
                m[:], xt.rearrange("p (c e) -> p c e", e=CH), axis=mybir.AxisListType.X
            )
            mx8 = sm.tile([P, 8], F32, tag="mx8")
            nc.vector.max(mx8[:], m[:])
            ci8 = sm.tile([P, 8], U32, tag="ci8")
            nc.vector.max_index(ci8[:], mx8[:], m[:])
            nc.vector.tensor_copy(cr_all[:, t : t + 1], ci8[:, 0:1])  # u32->f32
            gci = sm.tile([P, 1], F32, tag="gci")
            # global chunk id = (cr + t*128*NCH) + p*NCH
            nc.vector.tensor_scalar(
                gci[:],
                cr_all[:, t : t + 1],
                float(t * P * NCH),
                iota32[:],
                op0=mybir.AluOpType.add,
                op1=mybir.AluOpType.add,
            )
            nc.vector.tensor_copy(cidx_all[:, t : t + 1], gci[:])  # f32->i16

        # Phase B: gather each row's winning chunk (1KB) from x in HBM.
        scr_c = dr.tile([P, N_TILES], I16, tag="scr_c")
        idxw_c = keep.tile([P, ROWS // 16], I16, tag="idxw_c")
        _wrap_1024(nc, cidx_all, scr_c, idxw_c)
        wch = keep.tile([P, N_TILES, CH], F32)
        nc.gpsimd.dma_gather(
            wch[:],
            x.rearrange("r (c e) -> (r c) e", e=CH),
            idxw_c[:],
            num_idxs=ROWS,
            num_idxs_reg=ROWS,
            elem_size=CH,
        )

        # Phase C: offset within the winning chunk; final row-argmax.
        idx_all = keep.tile([P, N_TILES], I16)
        for t in range(N_TILES):
            wmx8 = sm.tile([P, 8], F32, tag="wmx8")
            nc.vector.max(wmx8[:], wch[:, t, :])
            wix8 = sm.tile([P, 8], U32, tag="wix8")
            nc.vector.max_index(wix8[:], wmx8[:], wch[:, t, :])
            wif = sm.tile([P, 1], F32, tag="wif")
            nc.vector.tensor_copy(wif[:], wix8[:, 0:1])  # u32->f32
            fin = sm.tile([P, 1], F32, tag="fin")
            # final = cr*CH + wi
            nc.vector.tensor_scalar(
                fin[:],
                cr_all[:, t : t + 1],
                float(CH),
                wif[:],
                op0=mybir.AluOpType.mult,
                op1=mybir.AluOpType.add,
            )
            nc.vector.tensor_copy(idx_all[:, t : t + 1], fin[:])  # f32->i16

        # Phase D: gather rows of W.T and write out.
        scr_w = dr.tile([P, N_TILES], I16, tag="scr_w")
        idxw_w = keep.tile([P, ROWS // 16], I16, tag="idxw_w")
        _wrap_1024(nc, idx_all, scr_w, idxw_w)
        wout = op.tile([P, N_TILES, OUT_DIM], F32)
        nc.gpsimd.dma_gather(
            wout[:],
            wt[:],
            idxw_w[:],
            num_idxs=ROWS,
            num_idxs_reg=ROWS,
            elem_size=OUT_DIM,
        )
        nc.sync.dma_start(y.rearrange("(t p) d -> p t d", p=P), wout[:])


def _emit_kernel_v3(tc: tile.TileContext, y: "bass.AP", x: "bass.AP", wt: "bass.AP"):
    """Fully per-tile pipelined hierarchical argmax + gather.

    Each [128, 8192] row-tile runs its complete chain (chunk-max reduce ->
    winning chunk -> 1KB/row chunk re-gather -> within-chunk argmax -> W.T row
    gather -> output store) independently, so the chains of tiles 0..6 hide
    under the HBM-bound loads of later tiles; only the last tile's ~20us chain
    sits on the critical path. Index wraps (partition redistribution into the
    dma_gather layout: idx j at partition j%16, slot j//16, replicated into
    partitions 16..31 for the second Q7 core of the queue) go through a DRAM
    scratch roundtrip issued on the otherwise-idle Scalar/Tensor sequencers.
    """
    nc = tc.nc
    with (
        tc.tile_pool(name="xp", bufs=3) as xp,
        tc.tile_pool(name="mp", bufs=3) as mp,
        tc.tile_pool(name="sm", bufs=3) as sm,
        tc.tile_pool(name="iw", bufs=3) as iw,
        tc.tile_pool(name="wc", bufs=3) as wc,
        tc.tile_pool(name="wo", bufs=3) as wo,
        tc.tile_pool(name="keep", bufs=1) as keep,
        tc.tile_pool(name="dr", bufs=3, space="DRAM") as dr,
    ):
        iota32 = keep.tile([P, 1], F32)  # p*NCH per partition
        nc.gpsimd.iota(
            iota32[:],
            pattern=[[0, 1]],
            base=0,
            channel_multiplier=NCH,
            allow_small_or_imprecise_dtypes=True,
        )

        for t in range(N_TILES):
            # ---- load + chunk maxes ----
            xt = xp.tile([P, QUANT_DIM], F32, tag="xt")
            nc.sync.dma_start(xt[:], x[t * P : (t + 1) * P, :])
            m = mp.tile([P, NCH], F32, tag="m")
            nc.vector.reduce_max(
                m[:], xt.rearrange("p (c e) -> p c e", e=CH), axis=mybir.AxisListType.X
            )
            mx8 = sm.tile([P, 8], F32, tag="mx8")
            nc.vector.max(mx8[:], m[:])
            ci8 = sm.tile([P, 8], U32, tag="ci8")
            nc.vector.max_index(ci8[:], mx8[:], m[:])
            crf = sm.tile([P, 1], F32, tag="crf")
            nc.vector.tensor_copy(crf[:], ci8[:, 0:1])  # u32 -> f32
            # chunk id within this tile's 4096 chunks: p*NCH + cr
            gci = sm.tile([P, 1], F32, tag="gci")
            nc.vector.tensor_scalar_add(gci[:], crf[:], iota32[:])
            gci16 = sm.tile([P, 1], I16, tag="gci16")
            nc.vector.tensor_copy(gci16[:], gci[:])  # f32 -> i16

            # ---- wrap chunk idx + 1KB/row chunk re-gather ----
            scr_c = dr.tile([P, 1], I16, tag="scr_c")
            nc.scalar.dma_start(scr_c[:], gci16[:])
            idxw_c = iw.tile([P, N_TILES], I16, tag="idxw_c")
            nc.gpsimd.memset(idxw_c[:], 0)
            src_c = scr_c.rearrange("(s1 q) one -> q (s1 one)", q=16)
            nc.scalar.dma_start(idxw_c[0:16, :], src_c)
            nc.scalar.dma_start(idxw_c[16:32, :], src_c)
            wch = wc.tile([P, 1, CH], F32, tag="wch")
            nc.gpsimd.dma_gather(
                wch[:],
                x[t * P : (t + 1) * P, :].rearrange("p (c e) -> (p c) e", e=CH),
                idxw_c[:],
                num_idxs=P,
                num_idxs_reg=P,
                elem_size=CH,
            )

            # ---- within-chunk offset; final row argmax ----
            wix8 = sm.tile([P, 8], U32, tag="wix8")
            nc.vector.max_index(wix8[:], mx8[:], wch[:, 0, :])
            wif = sm.tile([P, 1], F32, tag="wif")
            nc.vector.tensor_copy(wif[:], wix8[:, 0:1])  # u32 -> f32
            fin = sm.tile([P, 1], F32, tag="fin")
            nc.vector.tensor_scalar(
                fin[:],
                crf[:],
                float(CH),
                wif[:],
                op0=mybir.AluOpType.mult,
                op1=mybir.AluOpType.add,
            )
            fin16 = sm.tile([P, 1], I16, tag="fin16")
            nc.vector.tensor_copy(fin16[:], fin[:])  # f32 -> i16

            # ---- wrap final idx + gather W.T rows + store ----
            scr_w = dr.tile([P, 1], I16, tag="scr_w")
            nc.scalar.dma_start(scr_w[:], fin16[:])
            idxw_w = iw.tile([P, N_TILES], I16, tag="idxw_w")
            nc.gpsimd.memset(idxw_w[:], 0)
            src_w = scr_w.rearrange("(s1 q) one -> q (s1 one)", q=16)
            nc.scalar.dma_start(idxw_w[0:16, :], src_w)
            nc.scalar.dma_start(idxw_w[16:32, :], src_w)
            wout = wo.tile([P, 1, OUT_DIM], F32, tag="wout")
            nc.gpsimd.dma_gather(
                wout[:],
                wt[:],
                idxw_w[:],
                num_idxs=P,
                num_idxs_reg=P,
                elem_size=OUT_DIM,
            )
            nc.sync.dma_start(y[t * P : (t + 1) * P, :], wout[:, 0, :])


def _emit_kernel_v4(tc: tile.TileContext, y: "bass.AP", x: "bass.AP", wt: "bass.AP"):
    """Software-pipelined hierarchical argmax + gather.

    Same per-tile dataflow as v3, but emitted stage-major so each engine's
    in-order instruction stream never head-of-line blocks: all 8 HBM loads
    queue first on the SP HWDGE ring, the per-tile chains are interleaved with
    a 1-tile stagger (tile t's within-chunk stage emitted after tile t+1's
    chunk stage), and the output stores queue last.
    """
    nc = tc.nc
    with (
        tc.tile_pool(name="xp", bufs=4) as xp,
        tc.tile_pool(name="mp", bufs=3) as mp,
        tc.tile_pool(name="sm", bufs=3) as sm,
        tc.tile_pool(name="iw", bufs=3) as iw,
        tc.tile_pool(name="wc", bufs=3) as wc,
        tc.tile_pool(name="wo", bufs=8) as wo,
        tc.tile_pool(name="keep", bufs=1) as keep,
        tc.tile_pool(name="dr", bufs=3, space="DRAM") as dr,
    ):
        iota32 = keep.tile([P, 1], F32)  # p*NCH per partition
        nc.gpsimd.iota(
            iota32[:],
            pattern=[[0, 1]],
            base=0,
            channel_multiplier=NCH,
            allow_small_or_imprecise_dtypes=True,
        )

        # Stage 0: queue every HBM load up front (SP ring stays saturated).
        xts = []
        for t in range(N_TILES):
            xt = xp.tile([P, QUANT_DIM], F32, tag="xt")
            nc.sync.dma_start(xt[:], x[t * P : (t + 1) * P, :])
            xts.append(xt)

        crfs = [None] * N_TILES
        mx8s = [None] * N_TILES
        wchs = [None] * N_TILES
        wouts = [None] * N_TILES

        def stage_a(t):
            """chunk maxes -> winning chunk -> wrap -> 1KB/row chunk gather"""
            m = mp.tile([P, NCH], F32, tag="m")
            nc.vector.reduce_max(
                m[:],
                xts[t].rearrange("p (c e) -> p c e", e=CH),
                axis=mybir.AxisListType.X,
            )
            mx8 = sm.tile([P, 8], F32, tag="mx8")
            mx8s[t] = mx8
            nc.vector.max(mx8[:], m[:])
            ci8 = sm.tile([P, 8], U32, tag="ci8")
            nc.vector.max_index(ci8[:], mx8[:], m[:])
            crf = sm.tile([P, 1], F32, tag="crf")
            crfs[t] = crf
            nc.vector.tensor_copy(crf[:], ci8[:, 0:1])  # u32 -> f32
            gci = sm.tile([P, 1], F32, tag="gci")
            nc.vector.tensor_scalar_add(gci[:], crf[:], iota32[:])
            gci16 = sm.tile([P, 1], I16, tag="gci16")
            nc.vector.tensor_copy(gci16[:], gci[:])  # f32 -> i16

            scr_c = dr.tile([P, 1], I16, tag="scr_c")
            nc.scalar.dma_start(scr_c[:], gci16[:])
            idxw_c = iw.tile([P, N_TILES], I16, tag="idxw_c")
            nc.gpsimd.memset(idxw_c[:], 0)
            src_c = scr_c.rearrange("(s1 q) one -> q (s1 one)", q=16)
            nc.scalar.dma_start(idxw_c[0:16, :], src_c)
            nc.scalar.dma_start(idxw_c[16:32, :], src_c)
            wch = wc.tile([P, 1, CH], F32, tag="wch")
            wchs[t] = wch
            nc.gpsimd.dma_gather(
                wch[:],
                x[t * P : (t + 1) * P, :].rearrange("p (c e) -> (p c) e", e=CH),
                idxw_c[:],
                num_idxs=P,
                num_idxs_reg=P,
                elem_size=CH,
            )

        def stage_b(t):
            """within-chunk offset -> final idx -> wrap -> W.T row gather"""
            wix8 = sm.tile([P, 8], U32, tag="wix8")
            nc.vector.max_index(wix8[:], mx8s[t][:], wchs[t][:, 0, :])
            wif = sm.tile([P, 1], F32, tag="wif")
            nc.vector.tensor_copy(wif[:], wix8[:, 0:1])  # u32 -> f32
            fin = sm.tile([P, 1], F32, tag="fin")
            nc.vector.tensor_scalar(
                fin[:],
                crfs[t][:],
                float(CH),
                wif[:],
                op0=mybir.AluOpType.mult,
                op1=mybir.AluOpType.add,
            )
            fin16 = sm.tile([P, 1], I16, tag="fin16")
            nc.vector.tensor_copy(fin16[:], fin[:])  # f32 -> i16

            scr_w = dr.tile([P, 1], I16, tag="scr_w")
            nc.scalar.dma_start(scr_w[:], fin16[:])
            idxw_w = iw.tile([P, N_TILES], I16, tag="idxw_w")
            nc.gpsimd.memset(idxw_w[:], 0)
            src_w = scr_w.rearrange("(s1 q) one -> q (s1 one)", q=16)
            nc.scalar.dma_start(idxw_w[0:16, :], src_w)
            nc.scalar.dma_start(idxw_w[16:32, :], src_w)
            wout = wo.tile([P, 1, OUT_DIM], F32, tag="wout")
            wouts[t] = wout
            nc.gpsimd.dma_gather(
                wout[:],
                wt[:],
                idxw_w[:],
                num_idxs=P,
                num_idxs_reg=P,
                elem_size=OUT_DIM,
            )

        # 1-tile stagger: ... a(t), b(t-1), a(t+1), b(t) ...
        stage_a(0)
        for t in range(1, N_TILES):
            stage_a(t)
            stage_b(t - 1)
        stage_b(N_TILES - 1)

        # Stage Z: output stores, queued after the loads on the SP ring.
        for t in range(N_TILES):
            nc.sync.dma_start(y[t * P : (t + 1) * P, :], wouts[t][:, 0, :])


def _emit_kernel_v5(tc: tile.TileContext, y: "bass.AP", x: "bass.AP", wt: "bass.AP"):
    """Batched stage-major pipeline.

    All 8 HBM loads queue first and stream at full bandwidth; the per-row
    reduce/pick runs behind each load. Tail stages (index wrap -> chunk
    re-gather -> within-chunk argmax -> W.T gather -> store) run per BATCH of
    tiles: the first batch's tail hides under the second batch's loads, so only
    the last batch's ~30us tail sits on the critical path. Few, coarse DMAs
    keep the shared DMA-completion semaphore lanes from creating false
    cross-dependencies (which serialized the fine-grained variant).
    A dummy 16-row gather up front pre-loads the Q7 dma_gather ucode.
    """
    nc = tc.nc
    nb = len(BATCHES)
    with (
        tc.tile_pool(name="xp", bufs=4) as xp,
        tc.tile_pool(name="mp", bufs=3) as mp,
        tc.tile_pool(name="sm", bufs=4) as sm,
        tc.tile_pool(name="pk", bufs=N_TILES) as pk,
        tc.tile_pool(name="bt", bufs=2) as bt,
        tc.tile_pool(name="wc", bufs=2) as wc,
        tc.tile_pool(name="wo", bufs=2) as wo,
        tc.tile_pool(name="keep", bufs=1) as keep,
        tc.tile_pool(name="dr", bufs=2, space="DRAM") as dr,
    ):
        # Warm the Q7 dma_gather ucode while the first loads stream.
        widx = keep.tile([P, 1], I16)
        nc.gpsimd.memset(widx[:], 0)
        wscrap = keep.tile([P, 1, 64], F32)
        nc.gpsimd.dma_gather(
            wscrap[:],
            wt[:, 0:64],
            widx[:],
            num_idxs=16,
            num_idxs_reg=16,
            elem_size=64,
            elem_step=OUT_DIM,
        )

        # Stage 0: queue every HBM load up front.
        xts = []
        for t in range(N_TILES):
            xt = xp.tile([P, QUANT_DIM], F32, tag="xt")
            nc.sync.dma_start(xt[:], x[t * P : (t + 1) * P, :])
            xts.append(xt)

        iota32 = keep.tile([P, 1], F32)  # p*NCH per partition
        nc.gpsimd.iota(
            iota32[:],
            pattern=[[0, 1]],
            base=0,
            channel_multiplier=NCH,
            allow_small_or_imprecise_dtypes=True,
        )

        mx8s = [None] * N_TILES
        crfs = [None] * N_TILES

        def reduce_pick(t, i, cidx_b):
            """chunk maxes + winning chunk for tile t (column i of the batch)"""
            m = mp.tile([P, NCH], F32, tag="m")
            nc.vector.reduce_max(
                m[:],
                xts[t].rearrange("p (c e) -> p c e", e=CH),
                axis=mybir.AxisListType.X,
            )
            mx8 = pk.tile([P, 8], F32, tag="mx8")
            mx8s[t] = mx8
            nc.vector.max(mx8[:], m[:])
            ci8 = sm.tile([P, 8], U32, tag="ci8")
            nc.vector.max_index(ci8[:], mx8[:], m[:])
            crf = pk.tile([P, 1], F32, tag="crf")
            crfs[t] = crf
            nc.vector.tensor_copy(crf[:], ci8[:, 0:1])  # u32 -> f32
            gci = sm.tile([P, 1], F32, tag="gci")
            # chunk id within the batch's gather space: i*128*NCH + p*NCH + cr
            nc.vector.tensor_scalar(
                gci[:],
                crf[:],
                float(i * P * NCH),
                iota32[:],
                op0=mybir.AluOpType.add,
                op1=mybir.AluOpType.add,
            )
            nc.vector.tensor_copy(cidx_b[:, i : i + 1], gci[:])  # f32 -> i16
            return red

        def wrap(idx_b, n, scr_tag, idxw_tag, engine):
            """[128, n] i16 (value for row i*128+p at [p, i]) -> wrapped
            [128, 8n]: gather j reads partition j%16, slot j//16; replicated to
            partitions 16..31 for the queue's second Q7 core."""
            scr = dr.tile([P, n], I16, tag=scr_tag)
            engine.dma_start(scr[:], idx_b[:])
            idxw = bt.tile([P, 8 * n], I16, tag=idxw_tag)
            nc.gpsimd.memset(idxw[:], 0)
            src = scr.rearrange("(s1 q) i -> q i s1", q=16)
            dst0 = idxw[0:16, :].rearrange("q (i s1) -> q i s1", s1=8)
            dst1 = idxw[16:32, :].rearrange("q (i s1) -> q i s1", s1=8)
            engine.dma_start(dst0, src)
            engine.dma_start(dst1, src)
            return idxw

        def chunk_stage(b0, n, cidx_b):
            idxw_c = wrap(cidx_b, n, "scr_c", "idxw_c", nc.scalar)
            wch = wc.tile([P, n, CH], F32, tag="wch")
            nc.gpsimd.dma_gather(
                wch[:],
                x[b0 * P : (b0 + n) * P, :].rearrange("r (c e) -> (r c) e", e=CH),
                idxw_c[:],
                num_idxs=n * P,
                num_idxs_reg=n * P,
                elem_size=CH,
            )
            return wch

        def within(t, i, wch, fidx_b):
            """within-chunk offset -> final row argmax (column i of batch)"""
            wix8 = sm.tile([P, 8], U32, tag="wix8")
            nc.vector.max_index(wix8[:], mx8s[t][:], wch[:, i, :])
            wif = sm.tile([P, 1], F32, tag="wif")
            nc.vector.tensor_copy(wif[:], wix8[:, 0:1])  # u32 -> f32
            fin = sm.tile([P, 1], F32, tag="fin")
            nc.vector.tensor_scalar(
                fin[:],
                crfs[t][:],
                float(CH),
                wif[:],
                op0=mybir.AluOpType.mult,
                op1=mybir.AluOpType.add,
            )
            nc.vector.tensor_copy(fidx_b[:, i : i + 1], fin[:])  # f32 -> i16

        def out_stage(b0, n, fidx_b):
            idxw_w = wrap(fidx_b, n, "scr_w", "idxw_w", nc.scalar)
            wout = wo.tile([P, n, OUT_DIM], F32, tag="wout")
            nc.gpsimd.dma_gather(
                wout[:],
                wt[:],
                idxw_w[:],
                num_idxs=n * P,
                num_idxs_reg=n * P,
                elem_size=OUT_DIM,
            )
            nc.sync.dma_start(
                y[b0 * P : (b0 + n) * P, :].rearrange("(i p) d -> p i d", p=P),
                wout[:],
            )

        # Interleave: batch b's tail stages are emitted just after the first
        # reduce of batch b+1, so they hide under the remaining loads.
        pending = None  # (b0, n, cidx_b, wch-to-come...)
        for bi, (b0, n) in enumerate(BATCHES):
            cidx_b = bt.tile([P, n], I16, tag="cidx")
            for k in range(n):
                reduce_pick(b0 + k, k, cidx_b)
                if k == 0 and pending is not None:
                    pb0, pn, pcidx = pending
                    wch = chunk_stage(pb0, pn, pcidx)
                    fidx_b = bt.tile([P, pn], I16, tag="fidx")
                    for j in range(pn):
                        within(pb0 + j, j, wch, fidx_b)
                    out_stage(pb0, pn, fidx_b)
                    pending = None
            pending = (b0, n, cidx_b)

        pb0, pn, pcidx = pending
        wch = chunk_stage(pb0, pn, pcidx)
        fidx_b = bt.tile([P, pn], I16, tag="fidx")
        for j in range(pn):
            within(pb0 + j, j, wch, fidx_b)
        out_stage(pb0, pn, fidx_b)


def _emit_kernel_v6(tc: tile.TileContext, y: "bass.AP", x: "bass.AP", wt: "bass.AP"):
    """v5 + three fixes that came out of the v5 trace:

    - Index-wrap DMAs ride SWDGE (gpsimd.dma_start) instead of HWDGE: the 8
      HWDGE completion-semaphore lanes are shared round-robin, so a tiny wrap
      read could end up waiting on a still-running 4MB x load (observed ~20us
      false stalls).  SWDGE has its own lanes.
    - The W-row gathers run on SWDGE queue 1 (own Q7 core pair + ring), so
      their multi-MB transfers never head-of-line block the next batch's wrap
      writes/chunk gather on queue 0.  Queue 1's cores read the wrapped index
      buffer from partitions 32..63, CoreSim reads 0..15 - replicate to both.
    - The tail batches shrink (4/3/1) so the final batch's chain is minimal.
    """
    nc = tc.nc
    with (
        tc.tile_pool(name="xp", bufs=5) as xp,
        tc.tile_pool(name="mp", bufs=3) as mp,
        tc.tile_pool(name="sm", bufs=4) as sm,
        tc.tile_pool(name="pk", bufs=N_TILES) as pk,
        tc.tile_pool(name="bt", bufs=2) as bt,
        tc.tile_pool(name="wc", bufs=2) as wc,
        tc.tile_pool(name="wo", bufs=1) as wo,
        tc.tile_pool(name="keep", bufs=1) as keep,
        tc.tile_pool(name="dr", bufs=2, space="DRAM") as dr,
    ):
        # Warm the Q7 dma_gather ucode on both queues while loads stream.
        widx = keep.tile([P, 1], I16)
        nc.gpsimd.memset(widx[:], 0)
        for q in (0, WQ):
            wscrap = keep.tile([P, 1, 64], F32, tag=f"wscrap{q}")
            nc.gpsimd.dma_gather(
                wscrap[:],
                wt[:, 0:64],
                widx[:],
                num_idxs=16,
                num_idxs_reg=16,
                elem_size=64,
                elem_step=OUT_DIM,
                queue_num=q,
            )

        # Queue every HBM load up front.
        xts = []
        for t in range(N_TILES):
            xt = xp.tile([P, QUANT_DIM], F32, tag="xt")
            nc.sync.dma_start(xt[:], x[t * P : (t + 1) * P, :])
            xts.append(xt)

        iota32 = keep.tile([P, 1], F32)  # p*NCH per partition
        nc.gpsimd.iota(
            iota32[:],
            pattern=[[0, 1]],
            base=0,
            channel_multiplier=NCH,
            allow_small_or_imprecise_dtypes=True,
        )

        mx8s = [None] * N_TILES
        crfs = [None] * N_TILES

        def reduce_pick(t, i, cidx_b):
            m = mp.tile([P, NCH], F32, tag="m")
            red = nc.vector.reduce_max(
                m[:],
                xts[t].rearrange("p (c e) -> p c e", e=CH),
                axis=mybir.AxisListType.X,
            )
            mx8 = pk.tile([P, 8], F32, tag="mx8")
            mx8s[t] = mx8
            nc.vector.max(mx8[:], m[:])
            ci8 = sm.tile([P, 8], U32, tag="ci8")
            nc.vector.max_index(ci8[:], mx8[:], m[:])
            crf = pk.tile([P, 1], F32, tag="crf")
            crfs[t] = crf
            nc.vector.tensor_copy(crf[:], ci8[:, 0:1])  # u32 -> f32
            gci = sm.tile([P, 1], F32, tag="gci")
            nc.vector.tensor_scalar(
                gci[:],
                crf[:],
                float(i * P * NCH),
                iota32[:],
                op0=mybir.AluOpType.add,
                op1=mybir.AluOpType.add,
            )
            nc.vector.tensor_copy(cidx_b[:, i : i + 1], gci[:])  # f32 -> i16
            return red

        def wrap(idx_b, n, scr_tag, idxw_tag, groups, eng=None):
            """[128, n] i16 -> wrapped [128, 8n] via a DRAM roundtrip.
            Default engine is SWDGE (own completion-sem lanes, no false deps on
            in-flight HWDGE loads); the last batch uses scalar HWDGE (loads are
            done by then) to stay off the SWDGE ring behind big gathers.
            `groups` = 16-partition groups to fill (Q7 cores that will read)."""
            eng = eng or nc.gpsimd
            scr = dr.tile([P, n], I16, tag=scr_tag)
            eng.dma_start(scr[:], idx_b[:])
            idxw = bt.tile([P, 8 * n], I16, tag=idxw_tag)
            nc.gpsimd.memset(idxw[:], 0)
            src = scr.rearrange("(s1 q) i -> q i s1", q=16)
            for r in groups:
                dst = idxw[16 * r : 16 * (r + 1), :].rearrange(
                    "q (i s1) -> q i s1", s1=8
                )
                eng.dma_start(dst, src)
            return idxw

        def chunk_stage(b0, n, cidx_b, eng=None):
            idxw_c = wrap(cidx_b, n, "scr_c", "idxw_c", (0, 1), eng)
            wch = wc.tile([P, n, CH], F32, tag="wch")
            nc.gpsimd.dma_gather(
                wch[:],
                x[b0 * P : (b0 + n) * P, :].rearrange("r (c e) -> (r c) e", e=CH),
                idxw_c[:],
                num_idxs=n * P,
                num_idxs_reg=n * P,
                elem_size=CH,
            )
            return wch

        def within(t, i, wch, fidx_b, after=None):
            wix8 = sm.tile([P, 8], U32, tag="wix8")
            wix = nc.vector.max_index(wix8[:], mx8s[t][:], wch[:, i, :])
            if after is not None:
                # Keep this off the Vector stream until `after` has issued: the
                # scheduler's cost model underestimates the wrap+gather latency
                # and would otherwise park the stream here, stalling the
                # remaining reduces behind it (~25us on HW).
                tile.add_dep_helper(
                    wix.ins, after.ins, sync=False, reason="hold within behind reduce"
                )
            wif = sm.tile([P, 1], F32, tag="wif")
            nc.vector.tensor_copy(wif[:], wix8[:, 0:1])  # u32 -> f32
            fin = sm.tile([P, 1], F32, tag="fin")
            nc.vector.tensor_scalar(
                fin[:],
                crfs[t][:],
                float(CH),
                wif[:],
                op0=mybir.AluOpType.mult,
                op1=mybir.AluOpType.add,
            )
            nc.vector.tensor_copy(fidx_b[:, i : i + 1], fin[:])  # f32 -> i16

        def out_stage(b0, n, fidx_b, eng=None):
            wgroups = (0, 1) if WQ == 0 else (0, 2 * WQ, 2 * WQ + 1)
            idxw_w = wrap(fidx_b, n, "scr_w", "idxw_w", wgroups, eng)
            wout = wo.tile([P, n, OUT_DIM], F32, tag="wout")
            nc.gpsimd.dma_gather(
                wout[:],
                wt[:],
                idxw_w[:],
                num_idxs=n * P,
                num_idxs_reg=n * P,
                elem_size=OUT_DIM,
                queue_num=WQ,
            )
            nc.sync.dma_start(
                y[b0 * P : (b0 + n) * P, :].rearrange("(i p) d -> p i d", p=P),
                wout[:],
            )

        def emit_tail(b0, n, cidx_b, last=False):
            eng = nc.scalar if last else None
            wch = chunk_stage(b0, n, cidx_b, eng)
            fidx_b = bt.tile([P, n], I16, tag="fidx")
            for j in range(n):
                within(b0 + j, j, wch, fidx_b)
            out_stage(b0, n, fidx_b, eng)

        pending = None
        for b0, n in BATCHES:
            cidx_b = bt.tile([P, n], I16, tag="cidx")
            for k in range(n):
                reduce_pick(b0 + k, k, cidx_b)
                if k == 0 and pending is not None:
                    emit_tail(*pending)
                    pending = None
            pending = (b0, n, cidx_b)
        emit_tail(*pending, last=True)



def _emit_kernel_v7(tc: tile.TileContext, y: "bass.AP", x: "bass.AP", wt: "bass.AP"):
    """v6 helpers with a hand-scheduled emission for batches (4, 3, 1).

    Engine-stream plan (the Tile scheduler follows emission priority, so each
    engine's in-order stream must never park on a wait while later-ready work
    sits behind it):
      Vector: r0..r4 | r5 | within(b0) | r6 | r7 | within(b1) | within(b2)
      GpSimd: warmup, wrapC(b0), gatherC(b0), wrapW(b0), gatherW(b0),
              gatherC(b1), gatherC(b2), gatherW(b1), gatherW(b2)
      Scalar: wrapC(b1), wrapC(b2), wrapW(b1), wrapW(b2)   (HWDGE; loads are
              nearly drained by then so lane false-deps cost little)
      Sync:   loads 0..7, y(b0), y(b1), y(b2)
    b0's wraps ride SWDGE (loads still in flight -> HWDGE lanes unsafe); its
    2MB W-gather transfer finishes on the queue-0 ring before the later small
    wrap writes would need it, and the b1/b2 wraps avoid that ring entirely.
    """
    nc = tc.nc
    assert BATCHES == [(0, 4), (4, 3), (7, 1)]
    with (
        tc.tile_pool(name="xp", bufs=5) as xp,
        tc.tile_pool(name="mp", bufs=3) as mp,
        tc.tile_pool(name="sm", bufs=4) as sm,
        tc.tile_pool(name="pk", bufs=N_TILES) as pk,
        tc.tile_pool(name="bt", bufs=2) as bt,
        tc.tile_pool(name="wc", bufs=2) as wc,
        tc.tile_pool(name="wo", bufs=1) as wo,
        tc.tile_pool(name="keep", bufs=1) as keep,
        tc.tile_pool(name="dr", bufs=2, space="DRAM") as dr,
    ):
        # Warm the Q7 dma_gather ucode while the first loads stream.
        widx = keep.tile([P, 1], I16)
        nc.gpsimd.memset(widx[:], 0)
        wscrap = keep.tile([P, 1, 64], F32)
        nc.gpsimd.dma_gather(
            wscrap[:],
            wt[:, 0:64],
            widx[:],
            num_idxs=16,
            num_idxs_reg=16,
            elem_size=64,
            elem_step=OUT_DIM,
        )

        xts = []
        for t in range(N_TILES):
            xt = xp.tile([P, QUANT_DIM], F32, tag="xt")
            nc.sync.dma_start(xt[:], x[t * P : (t + 1) * P, :])
            xts.append(xt)

        iota32 = keep.tile([P, 1], F32)  # p*NCH per partition
        nc.gpsimd.iota(
            iota32[:],
            pattern=[[0, 1]],
            base=0,
            channel_multiplier=NCH,
            allow_small_or_imprecise_dtypes=True,
        )

        mx8s = [None] * N_TILES
        crfs = [None] * N_TILES

        def reduce_pick(t, i, cidx_b):
            m = mp.tile([P, NCH], F32, tag="m")
            red = nc.vector.reduce_max(
                m[:],
                xts[t].rearrange("p (c e) -> p c e", e=CH),
                axis=mybir.AxisListType.X,
            )
            mx8 = pk.tile([P, 8], F32, tag="mx8")
            mx8s[t] = mx8
            nc.vector.max(mx8[:], m[:])
            ci8 = sm.tile([P, 8], U32, tag="ci8")
            nc.vector.max_index(ci8[:], mx8[:], m[:])
            crf = pk.tile([P, 1], F32, tag="crf")
            crfs[t] = crf
            nc.vector.tensor_copy(crf[:], ci8[:, 0:1])  # u32 -> f32
            gci = sm.tile([P, 1], F32, tag="gci")
            nc.vector.tensor_scalar(
                gci[:],
                crf[:],
                float(i * P * NCH),
                iota32[:],
                op0=mybir.AluOpType.add,
                op1=mybir.AluOpType.add,
            )
            nc.vector.tensor_copy(cidx_b[:, i : i + 1], gci[:])  # f32 -> i16
            return red

        def wrap(idx_b, n, scr_tag, idxw_tag, eng):
            scr = dr.tile([P, n], I16, tag=scr_tag)
            eng.dma_start(scr[:], idx_b[:])
            idxw = bt.tile([P, 8 * n], I16, tag=idxw_tag)
            nc.gpsimd.memset(idxw[:], 0)
            src = scr.rearrange("(s1 q) i -> q i s1", q=16)
            for r in (0, 1):
                dst = idxw[16 * r : 16 * (r + 1), :].rearrange(
                    "q (i s1) -> q i s1", s1=8
                )
                eng.dma_start(dst, src)
            return idxw

        def gather_c(b0, n, idxw_c):
            wch = wc.tile([P, n, CH], F32, tag="wch")
            nc.gpsimd.dma_gather(
                wch[:],
                x[b0 * P : (b0 + n) * P, :].rearrange("r (c e) -> (r c) e", e=CH),
                idxw_c[:],
                num_idxs=n * P,
                num_idxs_reg=n * P,
                elem_size=CH,
            )
            return wch

        def within(t, i, wch, fidx_b, after=None):
            wix8 = sm.tile([P, 8], U32, tag="wix8")
            wix = nc.vector.max_index(wix8[:], mx8s[t][:], wch[:, i, :])
            if after is not None:
                # Keep this off the Vector stream until `after` has issued: the
                # scheduler's cost model underestimates the wrap+gather latency
                # and would otherwise park the stream here, stalling the
                # remaining reduces behind it (~25us on HW).
                tile.add_dep_helper(
                    wix.ins, after.ins, sync=False, reason="hold within behind reduce"
                )
            wif = sm.tile([P, 1], F32, tag="wif")
            nc.vector.tensor_copy(wif[:], wix8[:, 0:1])  # u32 -> f32
            fin = sm.tile([P, 1], F32, tag="fin")
            nc.vector.tensor_scalar(
                fin[:],
                crfs[t][:],
                float(CH),
                wif[:],
                op0=mybir.AluOpType.mult,
                op1=mybir.AluOpType.add,
            )
            nc.vector.tensor_copy(fidx_b[:, i : i + 1], fin[:])  # f32 -> i16

        def gather_w(n, idxw_w):
            wout = wo.tile([P, n, OUT_DIM], F32, tag="wout")
            nc.gpsimd.dma_gather(
                wout[:],
                wt[:],
                idxw_w[:],
                num_idxs=n * P,
                num_idxs_reg=n * P,
                elem_size=OUT_DIM,
            )
            return wout

        def store(b0, n, wout):
            nc.sync.dma_start(
                y[b0 * P : (b0 + n) * P, :].rearrange("(i p) d -> p i d", p=P),
                wout[:],
            )

        cidx = {}
        fidx = {}
        # b0 = tiles 0..3, b1 = tiles 4..6, b2 = tile 7
        cidx[0] = bt.tile([P, 4], I16, tag="cidx0", name="cidx0")
        cidx[1] = bt.tile([P, 3], I16, tag="cidx1", name="cidx1")
        cidx[2] = bt.tile([P, 1], I16, tag="cidx2", name="cidx2")
        reds = []
        for t in range(4):
            reds.append(reduce_pick(t, t, cidx[0]))
        reds.append(reduce_pick(4, 0, cidx[1]))
        idxw_c0 = wrap(cidx[0], 4, "scr_c0", "idxw_c0", nc.gpsimd)
        wch0 = gather_c(0, 4, idxw_c0)
        reds.append(reduce_pick(5, 1, cidx[1]))
        fidx[0] = bt.tile([P, 4], I16, tag="fidx0", name="fidx0")
        for j in range(4):
            within(j, j, wch0, fidx[0])
        idxw_w0 = wrap(fidx[0], 4, "scr_w0", "idxw_w0", nc.gpsimd)
        wout0 = gather_w(4, idxw_w0)
        store(0, 4, wout0)
        reduce_pick(6, 2, cidx[1])
        idxw_c1 = wrap(cidx[1], 3, "scr_c1", "idxw_c1", nc.scalar)
        wch1 = gather_c(4, 3, idxw_c1)
        reduce_pick(7, 0, cidx[2])
        idxw_c2 = wrap(cidx[2], 1, "scr_c2", "idxw_c2", nc.scalar)
        wch2 = gather_c(7, 1, idxw_c2)
        fidx[1] = bt.tile([P, 3], I16, tag="fidx1", name="fidx1")
        for j in range(3):
            within(4 + j, j, wch1, fidx[1])
        idxw_w1 = wrap(fidx[1], 3, "scr_w1", "idxw_w1", nc.scalar)
        wout1 = gather_w(3, idxw_w1)
        store(4, 3, wout1)
        fidx[2] = bt.tile([P, 1], I16, tag="fidx2", name="fidx2")
        within(7, 0, wch2, fidx[2])
        idxw_w2 = wrap(fidx[2], 1, "scr_w2", "idxw_w2", nc.scalar)
        wout2 = gather_w(1, idxw_w2)
        store(7, 1, wout2)


_CACHE: dict[str, object] = {}


def _build():
    if "nc" in _CACHE:
        return _CACHE["nc"]
    nc = bacc.Bacc(
        "TRN2",
        target_bir_lowering=False,
        debug=False,
        enable_asserts=True,
        num_swdge_queues=1,
    )
    x = nc.dram_tensor("x", [ROWS, QUANT_DIM], F32, kind="ExternalInput").ap()
    wt = nc.dram_tensor("wt", [QUANT_DIM, OUT_DIM], F32, kind="ExternalInput").ap()
    y = nc.dram_tensor("y", [ROWS, OUT_DIM], F32, kind="ExternalOutput").ap()
    emit = {
        1: _emit_kernel,
        2: _emit_kernel_v2,
        3: _emit_kernel_v3,
        4: _emit_kernel_v4,
        5: _emit_kernel_v5,
        6: _emit_kernel_v6,
        7: _emit_kernel_v7,
    }[VERSION]
    with tile.TileContext(nc) as tc:
        emit(tc, y, x, wt)
    nc.compile()
    _CACHE["nc"] = nc
    return nc


def kernel(x: np.ndarray, W: np.ndarray, **_unused) -> np.ndarray:
    assert x.shape == (N_TOKENS, QUANT_DIM) and W.shape == (OUT_DIM, QUANT_DIM)
    nc = _build()
    x = np.ascontiguousarray(x, dtype=np.float32)
    wt = np.ascontiguousarray(W.T.astype(np.float32, copy=False))
    in_maps = [
        {"x": x[i * ROWS : (i + 1) * ROWS], "wt": wt} for i in range(N_CORES)
    ]
    res = bass_utils.run_bass_kernel_spmd(nc, in_maps, core_ids=list(range(N_CORES)))
    return np.concatenate([res.results[i]["y"] for i in range(N_CORES)], axis=0)



# revision 4
# speedup vs baseline: 1.4354x; 1.4354x over previous
"""VQ codebook-lookup kernel for Trainium2 (8 NeuronCores, data-parallel).

Computes: idx = argmax(x, axis=-1); out = W.T[idx]  (i.e. out[n] = W[:, idx[n]])
  x: [8192, 8192] f32, W: [1024, 8192] f32 -> out: [8192, 1024] f32

Sharding: x split along tokens into 8 shards of [1024, 8192]; W.T replicated.
Each core: per-row argmax via VectorE max/max_index, then a DMA row-gather
from the transposed codebook in HBM.
"""

import sys

import numpy as np

sys.path.insert(0, "/opt/trn_rl_repo")

import concourse.bass as bass  # noqa: E402
import concourse.tile as tile  # noqa: E402
from concourse import bacc, bass_utils, mybir  # noqa: E402

N_CORES = 8
N_TOKENS = 8192
QUANT_DIM = 8192
OUT_DIM = 1024
ROWS = N_TOKENS // N_CORES  # rows (tokens) per core
P = 128  # SBUF partitions
N_TILES = ROWS // P  # row-tiles per core

F32 = mybir.dt.float32
I16 = mybir.dt.int16
I32 = mybir.dt.int32
U32 = mybir.dt.uint32

CH = 256  # chunk size for the hierarchical argmax
NCH = QUANT_DIM // CH  # 32 chunks per row

VERSION = 8
BATCHES = [(0, 4), (4, 3), (7, 1)]  # (first tile, n tiles) tail groups
WQ = 0  # single SWDGE queue (DMASW sem lanes are queue-locked; multi-queue trips Tile)


def _emit_kernel(tc: tile.TileContext, y: "bass.AP", x: "bass.AP", wt: "bass.AP"):
    """Per-core program. x: [ROWS, QUANT_DIM], wt: [QUANT_DIM, OUT_DIM] (=W.T),
    y: [ROWS, OUT_DIM]."""
    nc = tc.nc
    with (
        tc.tile_pool(name="xp", bufs=3) as xp,
        tc.tile_pool(name="sm", bufs=2 * N_TILES) as sm,
        tc.tile_pool(name="ip", bufs=1) as ip,
        tc.tile_pool(name="op", bufs=1) as op,
        tc.tile_pool(name="dr", bufs=1, space="DRAM") as dr,
    ):
        # Per-row argmax, one [128, QUANT_DIM] tile at a time.
        idx_all = ip.tile([P, N_TILES], I16)  # [row-in-tile, tile] argmax
        for t in range(N_TILES):
            xt = xp.tile([P, QUANT_DIM], F32)
            nc.sync.dma_start(xt[:], x[t * P : (t + 1) * P, :])
            mx = sm.tile([P, 8], F32, tag="mx")
            nc.vector.max(mx[:], xt[:])
            ix = sm.tile([P, 8], U32, tag="ix")
            nc.vector.max_index(ix[:], mx[:], xt[:])
            # argmax = ix[:, 0]; value < 8192 so the low half-word holds it.
            nc.vector.tensor_copy(idx_all[:, t : t + 1], ix.bitcast(I16)[:, 0:1])

        # dma_gather wants indices int16, "wrapped": gather j reads the index
        # at partition j%16, slot j//16 (replicated across the 8 gpsimd cores'
        # 16-partition groups). Row j = t*128 + p with p = s1*16 + q, so the
        # value for (q, slot=t*8+s1) is idx_all[s1*16+q, t]. Partition-crossing
        # shuffle goes through a DRAM scratch roundtrip.
        scratch = dr.tile([P, N_TILES], I16)
        nc.sync.dma_start(scratch[:], idx_all[:])
        idxw = ip.tile([P, ROWS // 16], I16)
        src = scratch.rearrange("(s1 q) t -> q t s1", q=16)
        for r in range(P // 16):
            dst = idxw[16 * r : 16 * (r + 1), :].rearrange(
                "q (t s1) -> q t s1", s1=N_TILES
            )
            nc.sync.dma_start(dst, src)

        # Gather rows of W.T from HBM: wout[p, t, :] = wt[idx[t*128+p], :]
        wout = op.tile([P, N_TILES, OUT_DIM], F32)
        nc.gpsimd.dma_gather(
            wout[:],
            wt[:],
            idxw[:],
            num_idxs=ROWS,
            num_idxs_reg=ROWS,
            elem_size=OUT_DIM,
        )
        nc.sync.dma_start(y.rearrange("(t p) d -> p t d", p=P), wout[:])


def _wrap_1024(nc, idx_sbuf, scratch, idxw):
    """Turn idx_sbuf [128, 8] int16 (value for row t*128+p at [p, t]) into the
    dma_gather wrapped layout idxw [128, 64]: gather j reads partition j%16,
    slot j//16; replicated across the 8 gpsimd-core partition groups.
    Row j = t*128 + s1*16 + q -> idxw[q, t*8+s1] = idx_sbuf[s1*16+q, t]."""
    nc.sync.dma_start(scratch[:], idx_sbuf[:])
    src = scratch.rearrange("(s1 q) t -> q t s1", q=16)
    for r in range(P // 16):
        dst = idxw[16 * r : 16 * (r + 1), :].rearrange(
            "q (t s1) -> q t s1", s1=N_TILES
        )
        nc.sync.dma_start(dst, src)


def _emit_kernel_v2(tc: tile.TileContext, y: "bass.AP", x: "bass.AP", wt: "bass.AP"):
    """Hierarchical argmax: one full pass computes per-chunk maxes (CH=256),
    cheap top-8 picks the winning chunk, a small HBM gather re-reads only the
    winning 1KB chunk per row, and a second tiny max/max_index finds the
    offset within it. ~1.06 passes of DVE work instead of 2."""
    nc = tc.nc
    with (
        tc.tile_pool(name="xp", bufs=3) as xp,
        tc.tile_pool(name="mp", bufs=N_TILES) as mp,
        tc.tile_pool(name="sm", bufs=2 * N_TILES) as sm,
        tc.tile_pool(name="keep", bufs=1) as keep,
        tc.tile_pool(name="op", bufs=1) as op,
        tc.tile_pool(name="dr", bufs=1, space="DRAM") as dr,
    ):
        # per-partition offsets: p*32 (global chunk id) as f32
        iota32 = keep.tile([P, 1], F32)
        nc.gpsimd.iota(
            iota32[:],
            pattern=[[0, 1]],
            base=0,
            channel_multiplier=NCH,
            allow_small_or_imprecise_dtypes=True,
        )

        cr_all = keep.tile([P, N_TILES], F32)  # winning chunk within row
        cidx_all = keep.tile([P, N_TILES], I16)  # global chunk id for gather
        # Phase A: chunk maxes + winning chunk per row.
        for t in range(N_TILES):
            xt = xp.tile([P, QUANT_DIM], F32)
            nc.sync.dma_start(xt[:], x[t * P : (t + 1) * P, :])
            m = mp.tile([P, NCH], F32, tag="m")
            nc.vector.reduce_max(
                m[:], xt.rearrange("p (c e) -> p c e", e=CH), axis=mybir.AxisListType.X
            )
            mx8 = sm.tile([P, 8], F32, tag="mx8")
            nc.vector.max(mx8[:], m[:])
            ci8 = sm.tile([P, 8], U32, tag="ci8")
            nc.vector.max_index(ci8[:], mx8[:], m[:])
            nc.vector.tensor_copy(cr_all[:, t : t + 1], ci8[:, 0:1])  # u32->f32
            gci = sm.tile([P, 1], F32, tag="gci")
            # global chunk id = (cr + t*128*NCH) + p*NCH
            nc.vector.tensor_scalar(
                gci[:],
                cr_all[:, t : t + 1],
                float(t * P * NCH),
                iota32[:],
                op0=mybir.AluOpType.add,
                op1=mybir.AluOpType.add,
            )
            nc.vector.tensor_copy(cidx_all[:, t : t + 1], gci[:])  # f32->i16

        # Phase B: gather each row's winning chunk (1KB) from x in HBM.
        scr_c = dr.tile([P, N_TILES], I16, tag="scr_c")
        idxw_c = keep.tile([P, ROWS // 16], I16, tag="idxw_c")
        _wrap_1024(nc, cidx_all, scr_c, idxw_c)
        wch = keep.tile([P, N_TILES, CH], F32)
        nc.gpsimd.dma_gather(
            wch[:],
            x.rearrange("r (c e) -> (r c) e", e=CH),
            idxw_c[:],
            num_idxs=ROWS,
            num_idxs_reg=ROWS,
            elem_size=CH,
        )

        # Phase C: offset within the winning chunk; final row-argmax.
        idx_all = keep.tile([P, N_TILES], I16)
        for t in range(N_TILES):
            wmx8 = sm.tile([P, 8], F32, tag="wmx8")
            nc.vector.max(wmx8[:], wch[:, t, :])
            wix8 = sm.tile([P, 8], U32, tag="wix8")
            nc.vector.max_index(wix8[:], wmx8[:], wch[:, t, :])
            wif = sm.tile([P, 1], F32, tag="wif")
            nc.vector.tensor_copy(wif[:], wix8[:, 0:1])  # u32->f32
            fin = sm.tile([P, 1], F32, tag="fin")
            # final = cr*CH + wi
            nc.vector.tensor_scalar(
                fin[:],
                cr_all[:, t : t + 1],
                float(CH),
                wif[:],
                op0=mybir.AluOpType.mult,
                op1=mybir.AluOpType.add,
            )
            nc.vector.tensor_copy(idx_all[:, t : t + 1], fin[:])  # f32->i16

        # Phase D: gather rows of W.T and write out.
        scr_w = dr.tile([P, N_TILES], I16, tag="scr_w")
        idxw_w = keep.tile([P, ROWS // 16], I16, tag="idxw_w")
        _wrap_1024(nc, idx_all, scr_w, idxw_w)
        wout = op.tile([P, N_TILES, OUT_DIM], F32)
        nc.gpsimd.dma_gather(
            wout[:],
            wt[:],
            idxw_w[:],
            num_idxs=ROWS,
            num_idxs_reg=ROWS,
            elem_size=OUT_DIM,
        )
        nc.sync.dma_start(y.rearrange("(t p) d -> p t d", p=P), wout[:])


def _emit_kernel_v3(tc: tile.TileContext, y: "bass.AP", x: "bass.AP", wt: "bass.AP"):
    """Fully per-tile pipelined hierarchical argmax + gather.

    Each [128, 8192] row-tile runs its complete chain (chunk-max reduce ->
    winning chunk -> 1KB/row chunk re-gather -> within-chunk argmax -> W.T row
    gather -> output store) independently, so the chains of tiles 0..6 hide
    under the HBM-bound loads of later tiles; only the last tile's ~20us chain
    sits on the critical path. Index wraps (partition redistribution into the
    dma_gather layout: idx j at partition j%16, slot j//16, replicated into
    partitions 16..31 for the second Q7 core of the queue) go through a DRAM
    scratch roundtrip issued on the otherwise-idle Scalar/Tensor sequencers.
    """
    nc = tc.nc
    with (
        tc.tile_pool(name="xp", bufs=3) as xp,
        tc.tile_pool(name="mp", bufs=3) as mp,
        tc.tile_pool(name="sm", bufs=3) as sm,
        tc.tile_pool(name="iw", bufs=3) as iw,
        tc.tile_pool(name="wc", bufs=3) as wc,
        tc.tile_pool(name="wo", bufs=3) as wo,
        tc.tile_pool(name="keep", bufs=1) as keep,
        tc.tile_pool(name="dr", bufs=3, space="DRAM") as dr,
    ):
        iota32 = keep.tile([P, 1], F32)  # p*NCH per partition
        nc.gpsimd.iota(
            iota32[:],
            pattern=[[0, 1]],
            base=0,
            channel_multiplier=NCH,
            allow_small_or_imprecise_dtypes=True,
        )

        for t in range(N_TILES):
            # ---- load + chunk maxes ----
            xt = xp.tile([P, QUANT_DIM], F32, tag="xt")
            nc.sync.dma_start(xt[:], x[t * P : (t + 1) * P, :])
            m = mp.tile([P, NCH], F32, tag="m")
            nc.vector.reduce_max(
                m[:], xt.rearrange("p (c e) -> p c e", e=CH), axis=mybir.AxisListType.X
            )
            mx8 = sm.tile([P, 8], F32, tag="mx8")
            nc.vector.max(mx8[:], m[:])
            ci8 = sm.tile([P, 8], U32, tag="ci8")
            nc.vector.max_index(ci8[:], mx8[:], m[:])
            crf = sm.tile([P, 1], F32, tag="crf")
            nc.vector.tensor_copy(crf[:], ci8[:, 0:1])  # u32 -> f32
            # chunk id within this tile's 4096 chunks: p*NCH + cr
            gci = sm.tile([P, 1], F32, tag="gci")
            nc.vector.tensor_scalar_add(gci[:], crf[:], iota32[:])
            gci16 = sm.tile([P, 1], I16, tag="gci16")
            nc.vector.tensor_copy(gci16[:], gci[:])  # f32 -> i16

            # ---- wrap chunk idx + 1KB/row chunk re-gather ----
            scr_c = dr.tile([P, 1], I16, tag="scr_c")
            nc.scalar.dma_start(scr_c[:], gci16[:])
            idxw_c = iw.tile([P, N_TILES], I16, tag="idxw_c")
            nc.gpsimd.memset(idxw_c[:], 0)
            src_c = scr_c.rearrange("(s1 q) one -> q (s1 one)", q=16)
            nc.scalar.dma_start(idxw_c[0:16, :], src_c)
            nc.scalar.dma_start(idxw_c[16:32, :], src_c)
            wch = wc.tile([P, 1, CH], F32, tag="wch")
            nc.gpsimd.dma_gather(
                wch[:],
                x[t * P : (t + 1) * P, :].rearrange("p (c e) -> (p c) e", e=CH),
                idxw_c[:],
                num_idxs=P,
                num_idxs_reg=P,
                elem_size=CH,
            )

            # ---- within-chunk offset; final row argmax ----
            wix8 = sm.tile([P, 8], U32, tag="wix8")
            nc.vector.max_index(wix8[:], mx8[:], wch[:, 0, :])
            wif = sm.tile([P, 1], F32, tag="wif")
            nc.vector.tensor_copy(wif[:], wix8[:, 0:1])  # u32 -> f32
            fin = sm.tile([P, 1], F32, tag="fin")
            nc.vector.tensor_scalar(
                fin[:],
                crf[:],
                float(CH),
                wif[:],
                op0=mybir.AluOpType.mult,
                op1=mybir.AluOpType.add,
            )
            fin16 = sm.tile([P, 1], I16, tag="fin16")
            nc.vector.tensor_copy(fin16[:], fin[:])  # f32 -> i16

            # ---- wrap final idx + gather W.T rows + store ----
            scr_w = dr.tile([P, 1], I16, tag="scr_w")
            nc.scalar.dma_start(scr_w[:], fin16[:])
            idxw_w = iw.tile([P, N_TILES], I16, tag="idxw_w")
            nc.gpsimd.memset(idxw_w[:], 0)
            src_w = scr_w.rearrange("(s1 q) one -> q (s1 one)", q=16)
            nc.scalar.dma_start(idxw_w[0:16, :], src_w)
            nc.scalar.dma_start(idxw_w[16:32, :], src_w)
            wout = wo.tile([P, 1, OUT_DIM], F32, tag="wout")
            nc.gpsimd.dma_gather(
                wout[:],
                wt[:],
                idxw_w[:],
                num_idxs=P,
                num_idxs_reg=P,
                elem_size=OUT_DIM,
            )
            nc.sync.dma_start(y[t * P : (t + 1) * P, :], wout[:, 0, :])


def _emit_kernel_v4(tc: tile.TileContext, y: "bass.AP", x: "bass.AP", wt: "bass.AP"):
    """Software-pipelined hierarchical argmax + gather.

    Same per-tile dataflow as v3, but emitted stage-major so each engine's
    in-order instruction stream never head-of-line blocks: all 8 HBM loads
    queue first on the SP HWDGE ring, the per-tile chains are interleaved with
    a 1-tile stagger (tile t's within-chunk stage emitted after tile t+1's
    chunk stage), and the output stores queue last.
    """
    nc = tc.nc
    with (
        tc.tile_pool(name="xp", bufs=4) as xp,
        tc.tile_pool(name="mp", bufs=3) as mp,
        tc.tile_pool(name="sm", bufs=3) as sm,
        tc.tile_pool(name="iw", bufs=3) as iw,
        tc.tile_pool(name="wc", bufs=3) as wc,
        tc.tile_pool(name="wo", bufs=8) as wo,
        tc.tile_pool(name="keep", bufs=1) as keep,
        tc.tile_pool(name="dr", bufs=3, space="DRAM") as dr,
    ):
        iota32 = keep.tile([P, 1], F32)  # p*NCH per partition
        nc.gpsimd.iota(
            iota32[:],
            pattern=[[0, 1]],
            base=0,
            channel_multiplier=NCH,
            allow_small_or_imprecise_dtypes=True,
        )

        # Stage 0: queue every HBM load up front (SP ring stays saturated).
        xts = []
        for t in range(N_TILES):
            xt = xp.tile([P, QUANT_DIM], F32, tag="xt")
            nc.sync.dma_start(xt[:], x[t * P : (t + 1) * P, :])
            xts.append(xt)

        crfs = [None] * N_TILES
        mx8s = [None] * N_TILES
        wchs = [None] * N_TILES
        wouts = [None] * N_TILES

        def stage_a(t):
            """chunk maxes -> winning chunk -> wrap -> 1KB/row chunk gather"""
            m = mp.tile([P, NCH], F32, tag="m")
            nc.vector.reduce_max(
                m[:],
                xts[t].rearrange("p (c e) -> p c e", e=CH),
                axis=mybir.AxisListType.X,
            )
            mx8 = sm.tile([P, 8], F32, tag="mx8")
            mx8s[t] = mx8
            nc.vector.max(mx8[:], m[:])
            ci8 = sm.tile([P, 8], U32, tag="ci8")
            nc.vector.max_index(ci8[:], mx8[:], m[:])
            crf = sm.tile([P, 1], F32, tag="crf")
            crfs[t] = crf
            nc.vector.tensor_copy(crf[:], ci8[:, 0:1])  # u32 -> f32
            gci = sm.tile([P, 1], F32, tag="gci")
            nc.vector.tensor_scalar_add(gci[:], crf[:], iota32[:])
            gci16 = sm.tile([P, 1], I16, tag="gci16")
            nc.vector.tensor_copy(gci16[:], gci[:])  # f32 -> i16

            scr_c = dr.tile([P, 1], I16, tag="scr_c")
            nc.scalar.dma_start(scr_c[:], gci16[:])
            idxw_c = iw.tile([P, N_TILES], I16, tag="idxw_c")
            nc.gpsimd.memset(idxw_c[:], 0)
            src_c = scr_c.rearrange("(s1 q) one -> q (s1 one)", q=16)
            nc.scalar.dma_start(idxw_c[0:16, :], src_c)
            nc.scalar.dma_start(idxw_c[16:32, :], src_c)
            wch = wc.tile([P, 1, CH], F32, tag="wch")
            wchs[t] = wch
            nc.gpsimd.dma_gather(
                wch[:],
                x[t * P : (t + 1) * P, :].rearrange("p (c e) -> (p c) e", e=CH),
                idxw_c[:],
                num_idxs=P,
                num_idxs_reg=P,
                elem_size=CH,
            )

        def stage_b(t):
            """within-chunk offset -> final idx -> wrap -> W.T row gather"""
            wix8 = sm.tile([P, 8], U32, tag="wix8")
            nc.vector.max_index(wix8[:], mx8s[t][:], wchs[t][:, 0, :])
            wif = sm.tile([P, 1], F32, tag="wif")
            nc.vector.tensor_copy(wif[:], wix8[:, 0:1])  # u32 -> f32
            fin = sm.tile([P, 1], F32, tag="fin")
            nc.vector.tensor_scalar(
                fin[:],
                crfs[t][:],
                float(CH),
                wif[:],
                op0=mybir.AluOpType.mult,
                op1=mybir.AluOpType.add,
            )
            fin16 = sm.tile([P, 1], I16, tag="fin16")
            nc.vector.tensor_copy(fin16[:], fin[:])  # f32 -> i16

            scr_w = dr.tile([P, 1], I16, tag="scr_w")
            nc.scalar.dma_start(scr_w[:], fin16[:])
            idxw_w = iw.tile([P, N_TILES], I16, tag="idxw_w")
            nc.gpsimd.memset(idxw_w[:], 0)
            src_w = scr_w.rearrange("(s1 q) one -> q (s1 one)", q=16)
            nc.scalar.dma_start(idxw_w[0:16, :], src_w)
            nc.scalar.dma_start(idxw_w[16:32, :], src_w)
            wout = wo.tile([P, 1, OUT_DIM], F32, tag="wout")
            wouts[t] = wout
            nc.gpsimd.dma_gather(
                wout[:],
                wt[:],
                idxw_w[:],
                num_idxs=P,
                num_idxs_reg=P,
                elem_size=OUT_DIM,
            )

        # 1-tile stagger: ... a(t), b(t-1), a(t+1), b(t) ...
        stage_a(0)
        for t in range(1, N_TILES):
            stage_a(t)
            stage_b(t - 1)
        stage_b(N_TILES - 1)

        # Stage Z: output stores, queued after the loads on the SP ring.
        for t in range(N_TILES):
            nc.sync.dma_start(y[t * P : (t + 1) * P, :], wouts[t][:, 0, :])


def _emit_kernel_v5(tc: tile.TileContext, y: "bass.AP", x: "bass.AP", wt: "bass.AP"):
    """Batched stage-major pipeline.

    All 8 HBM loads queue first and stream at full bandwidth; the per-row
    reduce/pick runs behind each load. Tail stages (index wrap -> chunk
    re-gather -> within-chunk argmax -> W.T gather -> store) run per BATCH of
    tiles: the first batch's tail hides under the second batch's loads, so only
    the last batch's ~30us tail sits on the critical path. Few, coarse DMAs
    keep the shared DMA-completion semaphore lanes from creating false
    cross-dependencies (which serialized the fine-grained variant).
    A dummy 16-row gather up front pre-loads the Q7 dma_gather ucode.
    """
    nc = tc.nc
    nb = len(BATCHES)
    with (
        tc.tile_pool(name="xp", bufs=4) as xp,
        tc.tile_pool(name="mp", bufs=3) as mp,
        tc.tile_pool(name="sm", bufs=4) as sm,
        tc.tile_pool(name="pk", bufs=N_TILES) as pk,
        tc.tile_pool(name="bt", bufs=2) as bt,
        tc.tile_pool(name="wc", bufs=2) as wc,
        tc.tile_pool(name="wo", bufs=2) as wo,
        tc.tile_pool(name="keep", bufs=1) as keep,
        tc.tile_pool(name="dr", bufs=2, space="DRAM") as dr,
    ):
        # Warm the Q7 dma_gather ucode while the first loads stream.
        widx = keep.tile([P, 1], I16)
        nc.gpsimd.memset(widx[:], 0)
        wscrap = keep.tile([P, 1, 64], F32)
        nc.gpsimd.dma_gather(
            wscrap[:],
            wt[:, 0:64],
            widx[:],
            num_idxs=16,
            num_idxs_reg=16,
            elem_size=64,
            elem_step=OUT_DIM,
        )

        # Stage 0: queue every HBM load up front.
        xts = []
        for t in range(N_TILES):
            xt = xp.tile([P, QUANT_DIM], F32, tag="xt")
            nc.sync.dma_start(xt[:], x[t * P : (t + 1) * P, :])
            xts.append(xt)

        iota32 = keep.tile([P, 1], F32)  # p*NCH per partition
        nc.gpsimd.iota(
            iota32[:],
            pattern=[[0, 1]],
            base=0,
            channel_multiplier=NCH,
            allow_small_or_imprecise_dtypes=True,
        )

        mx8s = [None] * N_TILES
        crfs = [None] * N_TILES

        def reduce_pick(t, i, cidx_b):
            """chunk maxes + winning chunk for tile t (column i of the batch)"""
            m = mp.tile([P, NCH], F32, tag="m")
            nc.vector.reduce_max(
                m[:],
                xts[t].rearrange("p (c e) -> p c e", e=CH),
                axis=mybir.AxisListType.X,
            )
            mx8 = pk.tile([P, 8], F32, tag="mx8")
            mx8s[t] = mx8
            nc.vector.max(mx8[:], m[:])
            ci8 = sm.tile([P, 8], U32, tag="ci8")
            nc.vector.max_index(ci8[:], mx8[:], m[:])
            crf = pk.tile([P, 1], F32, tag="crf")
            crfs[t] = crf
            nc.vector.tensor_copy(crf[:], ci8[:, 0:1])  # u32 -> f32
            gci = sm.tile([P, 1], F32, tag="gci")
            # chunk id within the batch's gather space: i*128*NCH + p*NCH + cr
            nc.vector.tensor_scalar(
                gci[:],
                crf[:],
                float(i * P * NCH),
                iota32[:],
                op0=mybir.AluOpType.add,
                op1=mybir.AluOpType.add,
            )
            nc.vector.tensor_copy(cidx_b[:, i : i + 1], gci[:])  # f32 -> i16
            return red

        def wrap(idx_b, n, scr_tag, idxw_tag, engine):
            """[128, n] i16 (value for row i*128+p at [p, i]) -> wrapped
            [128, 8n]: gather j reads partition j%16, slot j//16; replicated to
            partitions 16..31 for the queue's second Q7 core."""
            scr = dr.tile([P, n], I16, tag=scr_tag)
            engine.dma_start(scr[:], idx_b[:])
            idxw = bt.tile([P, 8 * n], I16, tag=idxw_tag)
            nc.gpsimd.memset(idxw[:], 0)
            src = scr.rearrange("(s1 q) i -> q i s1", q=16)
            dst0 = idxw[0:16, :].rearrange("q (i s1) -> q i s1", s1=8)
            dst1 = idxw[16:32, :].rearrange("q (i s1) -> q i s1", s1=8)
            engine.dma_start(dst0, src)
            engine.dma_start(dst1, src)
            return idxw

        def chunk_stage(b0, n, cidx_b):
            idxw_c = wrap(cidx_b, n, "scr_c", "idxw_c", nc.scalar)
            wch = wc.tile([P, n, CH], F32, tag="wch")
            nc.gpsimd.dma_gather(
                wch[:],
                x[b0 * P : (b0 + n) * P, :].rearrange("r (c e) -> (r c) e", e=CH),
                idxw_c[:],
                num_idxs=n * P,
                num_idxs_reg=n * P,
                elem_size=CH,
            )
            return wch

        def within(t, i, wch, fidx_b):
            """within-chunk offset -> final row argmax (column i of batch)"""
            wix8 = sm.tile([P, 8], U32, tag="wix8")
            nc.vector.max_index(wix8[:], mx8s[t][:], wch[:, i, :])
            wif = sm.tile([P, 1], F32, tag="wif")
            nc.vector.tensor_copy(wif[:], wix8[:, 0:1])  # u32 -> f32
            fin = sm.tile([P, 1], F32, tag="fin")
            nc.vector.tensor_scalar(
                fin[:],
                crfs[t][:],
                float(CH),
                wif[:],
                op0=mybir.AluOpType.mult,
                op1=mybir.AluOpType.add,
            )
            nc.vector.tensor_copy(fidx_b[:, i : i + 1], fin[:])  # f32 -> i16

        def out_stage(b0, n, fidx_b):
            idxw_w = wrap(fidx_b, n, "scr_w", "idxw_w", nc.scalar)
            wout = wo.tile([P, n, OUT_DIM], F32, tag="wout")
            nc.gpsimd.dma_gather(
                wout[:],
                wt[:],
                idxw_w[:],
                num_idxs=n * P,
                num_idxs_reg=n * P,
                elem_size=OUT_DIM,
            )
            nc.sync.dma_start(
                y[b0 * P : (b0 + n) * P, :].rearrange("(i p) d -> p i d", p=P),
                wout[:],
            )

        # Interleave: batch b's tail stages are emitted just after the first
        # reduce of batch b+1, so they hide under the remaining loads.
        pending = None  # (b0, n, cidx_b, wch-to-come...)
        for bi, (b0, n) in enumerate(BATCHES):
            cidx_b = bt.tile([P, n], I16, tag="cidx")
            for k in range(n):
                reduce_pick(b0 + k, k, cidx_b)
                if k == 0 and pending is not None:
                    pb0, pn, pcidx = pending
                    wch = chunk_stage(pb0, pn, pcidx)
                    fidx_b = bt.tile([P, pn], I16, tag="fidx")
                    for j in range(pn):
                        within(pb0 + j, j, wch, fidx_b)
                    out_stage(pb0, pn, fidx_b)
                    pending = None
            pending = (b0, n, cidx_b)

        pb0, pn, pcidx = pending
        wch = chunk_stage(pb0, pn, pcidx)
        fidx_b = bt.tile([P, pn], I16, tag="fidx")
        for j in range(pn):
            within(pb0 + j, j, wch, fidx_b)
        out_stage(pb0, pn, fidx_b)


def _emit_kernel_v6(tc: tile.TileContext, y: "bass.AP", x: "bass.AP", wt: "bass.AP"):
    """v5 + three fixes that came out of the v5 trace:

    - Index-wrap DMAs ride SWDGE (gpsimd.dma_start) instead of HWDGE: the 8
      HWDGE completion-semaphore lanes are shared round-robin, so a tiny wrap
      read could end up waiting on a still-running 4MB x load (observed ~20us
      false stalls).  SWDGE has its own lanes.
    - The W-row gathers run on SWDGE queue 1 (own Q7 core pair + ring), so
      their multi-MB transfers never head-of-line block the next batch's wrap
      writes/chunk gather on queue 0.  Queue 1's cores read the wrapped index
      buffer from partitions 32..63, CoreSim reads 0..15 - replicate to both.
    - The tail batches shrink (4/3/1) so the final batch's chain is minimal.
    """
    nc = tc.nc
    with (
        tc.tile_pool(name="xp", bufs=5) as xp,
        tc.tile_pool(name="mp", bufs=3) as mp,
        tc.tile_pool(name="sm", bufs=4) as sm,
        tc.tile_pool(name="pk", bufs=N_TILES) as pk,
        tc.tile_pool(name="bt", bufs=2) as bt,
        tc.tile_pool(name="wc", bufs=2) as wc,
        tc.tile_pool(name="wo", bufs=1) as wo,
        tc.tile_pool(name="keep", bufs=1) as keep,
        tc.tile_pool(name="dr", bufs=2, space="DRAM") as dr,
    ):
        # Warm the Q7 dma_gather ucode on both queues while loads stream.
        widx = keep.tile([P, 1], I16)
        nc.gpsimd.memset(widx[:], 0)
        for q in (0, WQ):
            wscrap = keep.tile([P, 1, 64], F32, tag=f"wscrap{q}")
            nc.gpsimd.dma_gather(
                wscrap[:],
                wt[:, 0:64],
                widx[:],
                num_idxs=16,
                num_idxs_reg=16,
                elem_size=64,
                elem_step=OUT_DIM,
                queue_num=q,
            )

        # Queue every HBM load up front.
        xts = []
        for t in range(N_TILES):
            xt = xp.tile([P, QUANT_DIM], F32, tag="xt")
            nc.sync.dma_start(xt[:], x[t * P : (t + 1) * P, :])
            xts.append(xt)

        iota32 = keep.tile([P, 1], F32)  # p*NCH per partition
        nc.gpsimd.iota(
            iota32[:],
            pattern=[[0, 1]],
            base=0,
            channel_multiplier=NCH,
            allow_small_or_imprecise_dtypes=True,
        )

        mx8s = [None] * N_TILES
        crfs = [None] * N_TILES

        def reduce_pick(t, i, cidx_b):
            m = mp.tile([P, NCH], F32, tag="m")
            red = nc.vector.reduce_max(
                m[:],
                xts[t].rearrange("p (c e) -> p c e", e=CH),
                axis=mybir.AxisListType.X,
            )
            mx8 = pk.tile([P, 8], F32, tag="mx8")
            mx8s[t] = mx8
            nc.vector.max(mx8[:], m[:])
            ci8 = sm.tile([P, 8], U32, tag="ci8")
            nc.vector.max_index(ci8[:], mx8[:], m[:])
            crf = pk.tile([P, 1], F32, tag="crf")
            crfs[t] = crf
            nc.vector.tensor_copy(crf[:], ci8[:, 0:1])  # u32 -> f32
            gci = sm.tile([P, 1], F32, tag="gci")
            nc.vector.tensor_scalar(
                gci[:],
                crf[:],
                float(i * P * NCH),
                iota32[:],
                op0=mybir.AluOpType.add,
                op1=mybir.AluOpType.add,
            )
            nc.vector.tensor_copy(cidx_b[:, i : i + 1], gci[:])  # f32 -> i16
            return red

        def wrap(idx_b, n, scr_tag, idxw_tag, groups, eng=None):
            """[128, n] i16 -> wrapped [128, 8n] via a DRAM roundtrip.
            Default engine is SWDGE (own completion-sem lanes, no false deps on
            in-flight HWDGE loads); the last batch uses scalar HWDGE (loads are
            done by then) to stay off the SWDGE ring behind big gathers.
            `groups` = 16-partition groups to fill (Q7 cores that will read)."""
            eng = eng or nc.gpsimd
            scr = dr.tile([P, n], I16, tag=scr_tag)
            eng.dma_start(scr[:], idx_b[:])
            idxw = bt.tile([P, 8 * n], I16, tag=idxw_tag)
            nc.gpsimd.memset(idxw[:], 0)
            src = scr.rearrange("(s1 q) i -> q i s1", q=16)
            for r in groups:
                dst = idxw[16 * r : 16 * (r + 1), :].rearrange(
                    "q (i s1) -> q i s1", s1=8
                )
                eng.dma_start(dst, src)
            return idxw

        def chunk_stage(b0, n, cidx_b, eng=None):
            idxw_c = wrap(cidx_b, n, "scr_c", "idxw_c", (0, 1), eng)
            wch = wc.tile([P, n, CH], F32, tag="wch")
            nc.gpsimd.dma_gather(
                wch[:],
                x[b0 * P : (b0 + n) * P, :].rearrange("r (c e) -> (r c) e", e=CH),
                idxw_c[:],
                num_idxs=n * P,
                num_idxs_reg=n * P,
                elem_size=CH,
            )
            return wch

        def within(t, i, wch, fidx_b, after=None):
            wix8 = sm.tile([P, 8], U32, tag="wix8")
            wix = nc.vector.max_index(wix8[:], mx8s[t][:], wch[:, i, :])
            if after is not None:
                # Keep this off the Vector stream until `after` has issued: the
                # scheduler's cost model underestimates the wrap+gather latency
                # and would otherwise park the stream here, stalling the
                # remaining reduces behind it (~25us on HW).
                tile.add_dep_helper(
                    wix.ins, after.ins, sync=False, reason="hold within behind reduce"
                )
            wif = sm.tile([P, 1], F32, tag="wif")
            nc.vector.tensor_copy(wif[:], wix8[:, 0:1])  # u32 -> f32
            fin = sm.tile([P, 1], F32, tag="fin")
            nc.vector.tensor_scalar(
                fin[:],
                crfs[t][:],
                float(CH),
                wif[:],
                op0=mybir.AluOpType.mult,
                op1=mybir.AluOpType.add,
            )
            nc.vector.tensor_copy(fidx_b[:, i : i + 1], fin[:])  # f32 -> i16

        def out_stage(b0, n, fidx_b, eng=None):
            wgroups = (0, 1) if WQ == 0 else (0, 2 * WQ, 2 * WQ + 1)
            idxw_w = wrap(fidx_b, n, "scr_w", "idxw_w", wgroups, eng)
            wout = wo.tile([P, n, OUT_DIM], F32, tag="wout")
            nc.gpsimd.dma_gather(
                wout[:],
                wt[:],
                idxw_w[:],
                num_idxs=n * P,
                num_idxs_reg=n * P,
                elem_size=OUT_DIM,
                queue_num=WQ,
            )
            nc.sync.dma_start(
                y[b0 * P : (b0 + n) * P, :].rearrange("(i p) d -> p i d", p=P),
                wout[:],
            )

        def emit_tail(b0, n, cidx_b, last=False):
            eng = nc.scalar if last else None
            wch = chunk_stage(b0, n, cidx_b, eng)
            fidx_b = bt.tile([P, n], I16, tag="fidx")
            for j in range(n):
                within(b0 + j, j, wch, fidx_b)
            out_stage(b0, n, fidx_b, eng)

        pending = None
        for b0, n in BATCHES:
            cidx_b = bt.tile([P, n], I16, tag="cidx")
            for k in range(n):
                reduce_pick(b0 + k, k, cidx_b)
                if k == 0 and pending is not None:
                    emit_tail(*pending)
                    pending = None
            pending = (b0, n, cidx_b)
        emit_tail(*pending, last=True)



def _emit_kernel_v7(tc: tile.TileContext, y: "bass.AP", x: "bass.AP", wt: "bass.AP"):
    """v6 helpers with a hand-scheduled emission for batches (4, 3, 1).

    Engine-stream plan (the Tile scheduler follows emission priority, so each
    engine's in-order stream must never park on a wait while later-ready work
    sits behind it):
      Vector: r0..r4 | r5 | within(b0) | r6 | r7 | within(b1) | within(b2)
      GpSimd: warmup, wrapC(b0), gatherC(b0), wrapW(b0), gatherW(b0),
              gatherC(b1), gatherC(b2), gatherW(b1), gatherW(b2)
      Scalar: wrapC(b1), wrapC(b2), wrapW(b1), wrapW(b2)   (HWDGE; loads are
              nearly drained by then so lane false-deps cost little)
      Sync:   loads 0..7, y(b0), y(b1), y(b2)
    b0's wraps ride SWDGE (loads still in flight -> HWDGE lanes unsafe); its
    2MB W-gather transfer finishes on the queue-0 ring before the later small
    wrap writes would need it, and the b1/b2 wraps avoid that ring entirely.
    """
    nc = tc.nc
    assert BATCHES == [(0, 4), (4, 3), (7, 1)]
    with (
        tc.tile_pool(name="xp", bufs=5) as xp,
        tc.tile_pool(name="mp", bufs=3) as mp,
        tc.tile_pool(name="sm", bufs=4) as sm,
        tc.tile_pool(name="pk", bufs=N_TILES) as pk,
        tc.tile_pool(name="bt", bufs=2) as bt,
        tc.tile_pool(name="wc", bufs=2) as wc,
        tc.tile_pool(name="wo", bufs=1) as wo,
        tc.tile_pool(name="keep", bufs=1) as keep,
        tc.tile_pool(name="dr", bufs=2, space="DRAM") as dr,
    ):
        # Warm the Q7 dma_gather ucode while the first loads stream.
        widx = keep.tile([P, 1], I16)
        nc.gpsimd.memset(widx[:], 0)
        wscrap = keep.tile([P, 1, 64], F32)
        nc.gpsimd.dma_gather(
            wscrap[:],
            wt[:, 0:64],
            widx[:],
            num_idxs=16,
            num_idxs_reg=16,
            elem_size=64,
            elem_step=OUT_DIM,
        )

        xts = []
        for t in range(N_TILES):
            xt = xp.tile([P, QUANT_DIM], F32, tag="xt")
            nc.sync.dma_start(xt[:], x[t * P : (t + 1) * P, :])
            xts.append(xt)

        iota32 = keep.tile([P, 1], F32)  # p*NCH per partition
        nc.gpsimd.iota(
            iota32[:],
            pattern=[[0, 1]],
            base=0,
            channel_multiplier=NCH,
            allow_small_or_imprecise_dtypes=True,
        )

        mx8s = [None] * N_TILES
        crfs = [None] * N_TILES

        def reduce_pick(t, i, cidx_b):
            m = mp.tile([P, NCH], F32, tag="m")
            red = nc.vector.reduce_max(
                m[:],
                xts[t].rearrange("p (c e) -> p c e", e=CH),
                axis=mybir.AxisListType.X,
            )
            mx8 = pk.tile([P, 8], F32, tag="mx8")
            mx8s[t] = mx8
            nc.vector.max(mx8[:], m[:])
            ci8 = sm.tile([P, 8], U32, tag="ci8")
            nc.vector.max_index(ci8[:], mx8[:], m[:])
            crf = pk.tile([P, 1], F32, tag="crf")
            crfs[t] = crf
            nc.vector.tensor_copy(crf[:], ci8[:, 0:1])  # u32 -> f32
            gci = sm.tile([P, 1], F32, tag="gci")
            nc.vector.tensor_scalar(
                gci[:],
                crf[:],
                float(i * P * NCH),
                iota32[:],
                op0=mybir.AluOpType.add,
                op1=mybir.AluOpType.add,
            )
            nc.vector.tensor_copy(cidx_b[:, i : i + 1], gci[:])  # f32 -> i16
            return red

        def wrap(idx_b, n, scr_tag, idxw_tag, eng):
            scr = dr.tile([P, n], I16, tag=scr_tag)
            eng.dma_start(scr[:], idx_b[:])
            idxw = bt.tile([P, 8 * n], I16, tag=idxw_tag)
            nc.gpsimd.memset(idxw[:], 0)
            src = scr.rearrange("(s1 q) i -> q i s1", q=16)
            for r in (0, 1):
                dst = idxw[16 * r : 16 * (r + 1), :].rearrange(
                    "q (i s1) -> q i s1", s1=8
                )
                eng.dma_start(dst, src)
            return idxw

        def gather_c(b0, n, idxw_c):
            wch = wc.tile([P, n, CH], F32, tag="wch")
            nc.gpsimd.dma_gather(
                wch[:],
                x[b0 * P : (b0 + n) * P, :].rearrange("r (c e) -> (r c) e", e=CH),
                idxw_c[:],
                num_idxs=n * P,
                num_idxs_reg=n * P,
                elem_size=CH,
            )
            return wch

        def within(t, i, wch, fidx_b, after=None):
            wix8 = sm.tile([P, 8], U32, tag="wix8")
            wix = nc.vector.max_index(wix8[:], mx8s[t][:], wch[:, i, :])
            if after is not None:
                # Keep this off the Vector stream until `after` has issued: the
                # scheduler's cost model underestimates the wrap+gather latency
                # and would otherwise park the stream here, stalling the
                # remaining reduces behind it (~25us on HW).
                tile.add_dep_helper(
                    wix.ins, after.ins, sync=False, reason="hold within behind reduce"
                )
            wif = sm.tile([P, 1], F32, tag="wif")
            nc.vector.tensor_copy(wif[:], wix8[:, 0:1])  # u32 -> f32
            fin = sm.tile([P, 1], F32, tag="fin")
            nc.vector.tensor_scalar(
                fin[:],
                crfs[t][:],
                float(CH),
                wif[:],
                op0=mybir.AluOpType.mult,
                op1=mybir.AluOpType.add,
            )
            nc.vector.tensor_copy(fidx_b[:, i : i + 1], fin[:])  # f32 -> i16

        def gather_w(n, idxw_w):
            wout = wo.tile([P, n, OUT_DIM], F32, tag="wout")
            nc.gpsimd.dma_gather(
                wout[:],
                wt[:],
                idxw_w[:],
                num_idxs=n * P,
                num_idxs_reg=n * P,
                elem_size=OUT_DIM,
            )
            return wout

        def store(b0, n, wout):
            nc.sync.dma_start(
                y[b0 * P : (b0 + n) * P, :].rearrange("(i p) d -> p i d", p=P),
                wout[:],
            )

        cidx = {}
        fidx = {}
        # b0 = tiles 0..3, b1 = tiles 4..6, b2 = tile 7
        cidx[0] = bt.tile([P, 4], I16, tag="cidx0", name="cidx0")
        cidx[1] = bt.tile([P, 3], I16, tag="cidx1", name="cidx1")
        cidx[2] = bt.tile([P, 1], I16, tag="cidx2", name="cidx2")
        reds = []
        for t in range(4):
            reds.append(reduce_pick(t, t, cidx[0]))
        reds.append(reduce_pick(4, 0, cidx[1]))
        idxw_c0 = wrap(cidx[0], 4, "scr_c0", "idxw_c0", nc.gpsimd)
        wch0 = gather_c(0, 4, idxw_c0)
        reds.append(reduce_pick(5, 1, cidx[1]))
        fidx[0] = bt.tile([P, 4], I16, tag="fidx0", name="fidx0")
        for j in range(4):
            within(j, j, wch0, fidx[0])
        idxw_w0 = wrap(fidx[0], 4, "scr_w0", "idxw_w0", nc.gpsimd)
        wout0 = gather_w(4, idxw_w0)
        store(0, 4, wout0)
        reduce_pick(6, 2, cidx[1])
        idxw_c1 = wrap(cidx[1], 3, "scr_c1", "idxw_c1", nc.scalar)
        wch1 = gather_c(4, 3, idxw_c1)
        reduce_pick(7, 0, cidx[2])
        idxw_c2 = wrap(cidx[2], 1, "scr_c2", "idxw_c2", nc.scalar)
        wch2 = gather_c(7, 1, idxw_c2)
        fidx[1] = bt.tile([P, 3], I16, tag="fidx1", name="fidx1")
        for j in range(3):
            within(4 + j, j, wch1, fidx[1])
        idxw_w1 = wrap(fidx[1], 3, "scr_w1", "idxw_w1", nc.scalar)
        wout1 = gather_w(3, idxw_w1)
        store(4, 3, wout1)
        fidx[2] = bt.tile([P, 1], I16, tag="fidx2", name="fidx2")
        within(7, 0, wch2, fidx[2])
        idxw_w2 = wrap(fidx[2], 1, "scr_w2", "idxw_w2", nc.scalar)
        wout2 = gather_w(1, idxw_w2)
        store(7, 1, wout2)


def _emit_kernel_v8(tc: tile.TileContext, y: "bass.AP", x: "bass.AP", wt: "bass.AP"):
    """Per-tile pipeline built on indirect_dma_start (natural [P,1] indices).

    v7's tail cost came from the dma_gather index-wrap machinery: every batch
    paid a DRAM scratch roundtrip x2 (25us completion latencies under load
    traffic) plus 0xf0 ucode dispatches, and batch tails were gated on the
    LAST tile of the batch, so ~85us of serial tail ran after the final load.

    v8 drops dma_gather entirely. Per 128-row tile:
      reduce_max -> chunk maxes m[P,32] -> max/max_index pick the winning
      chunk -> indirect_dma_start re-gathers each row's winning 1KB chunk
      (offsets straight from SBUF [P,1] i32 - no wrap, no scratch) ->
      max_index within the chunk -> indirect_dma_start gathers W.T rows ->
      store. Tiles pipeline independently; only the last tile's ~18us chain
      trails the final load, and tile 7 is loaded in 3 column pieces so its
      final reduce is 2.2us instead of 8.7us.

    Engine streams: Sync = x loads only; Scalar = y stores; GpSimd = the two
    indirect gathers per tile; Vector = reduce/pick/find with finds staggered
    one tile behind reduces so the in-order stream never parks on a gather.
    """
    nc = tc.nc
    with (
        tc.tile_pool(name="xp", bufs=5) as xp,
        tc.tile_pool(name="mp", bufs=2) as mp,
        tc.tile_pool(name="sm", bufs=3) as sm,
        tc.tile_pool(name="pk", bufs=N_TILES) as pk,
        tc.tile_pool(name="ii", bufs=3) as ii,
        tc.tile_pool(name="wc", bufs=3) as wc,
        tc.tile_pool(name="fi", bufs=3) as fi,
        tc.tile_pool(name="wo", bufs=3) as wo,
        tc.tile_pool(name="keep", bufs=1) as keep,
    ):
        iota32 = keep.tile([P, 1], F32)  # p*NCH per partition
        nc.gpsimd.iota(
            iota32[:],
            pattern=[[0, 1]],
            base=0,
            channel_multiplier=NCH,
            allow_small_or_imprecise_dtypes=True,
        )

        # x viewed as a flat chunk table for the winning-chunk re-gather.
        x_chunks = x.rearrange("r (c e) -> (r c) e", e=CH)

        # Queue every HBM load up front on the Sync HWDGE ring. Tile 7 goes
        # in three column pieces so the tail's final reduce is small.
        T_LAST = N_TILES - 1
        xts = []
        for t in range(T_LAST):
            xt = xp.tile([P, QUANT_DIM], F32, tag="xt")
            nc.sync.dma_start(xt[:], x[t * P : (t + 1) * P, :])
            xts.append(xt)
        xt7 = xp.tile([P, QUANT_DIM], F32, tag="xt")
        xts.append(xt7)
        PIECES = [(0, 4096), (4096, 6144), (6144, 8192)]
        for c0, c1 in PIECES:
            nc.sync.dma_start(
                xt7[:, c0:c1], x[T_LAST * P : (T_LAST + 1) * P, c0:c1]
            )

        mx8s = [None] * N_TILES
        crfs = [None] * N_TILES
        cidxs = [None] * N_TILES
        wchs = [None] * N_TILES
        fidxs = [None] * N_TILES
        wouts = [None] * N_TILES
        m7 = None

        def reduce_t(t):
            m = mp.tile([P, NCH], F32, tag="m")
            nc.vector.reduce_max(
                m[:],
                xts[t].rearrange("p (c e) -> p c e", e=CH),
                axis=mybir.AxisListType.X,
            )
            return m

        def pick_t(t, m):
            """winning chunk + global chunk id for the re-gather"""
            mx8 = pk.tile([P, 8], F32, tag="mx8")
            mx8s[t] = mx8
            nc.vector.max(mx8[:], m[:])
            ci8 = sm.tile([P, 8], U32, tag="ci8")
            nc.vector.max_index(ci8[:], mx8[:], m[:])
            crf = pk.tile([P, 1], F32, tag="crf")
            crfs[t] = crf
            nc.vector.tensor_copy(crf[:], ci8[:, 0:1])  # u32 -> f32
            gci = sm.tile([P, 1], F32, tag="gci")
            # chunk id in x_chunks: (t*128 + p)*NCH + cr
            nc.vector.tensor_scalar(
                gci[:],
                crf[:],
                float(t * P * NCH),
                iota32[:],
                op0=mybir.AluOpType.add,
                op1=mybir.AluOpType.add,
            )
            cidx = ii.tile([P, 1], I32, tag="cidx")
            cidxs[t] = cidx
            nc.vector.tensor_copy(cidx[:], gci[:])  # f32 -> i32

        def chunk_gather(t):
            wch = wc.tile([P, CH], F32, tag="wch")
            wchs[t] = wch
            nc.gpsimd.indirect_dma_start(
                out=wch[:],
                out_offset=None,
                in_=x_chunks,
                in_offset=bass.IndirectOffsetOnAxis(ap=cidxs[t][:, :1], axis=0),
            )

        def find_t(t):
            """offset within the winning chunk -> final row argmax"""
            wix8 = sm.tile([P, 8], U32, tag="wix8")
            nc.vector.max_index(wix8[:], mx8s[t][:], wchs[t][:])
            wif = sm.tile([P, 1], F32, tag="wif")
            nc.vector.tensor_copy(wif[:], wix8[:, 0:1])  # u32 -> f32
            fin = sm.tile([P, 1], F32, tag="fin")
            nc.vector.tensor_scalar(
                fin[:],
                crfs[t][:],
                float(CH),
                wif[:],
                op0=mybir.AluOpType.mult,
                op1=mybir.AluOpType.add,
            )
            fidx = fi.tile([P, 1], I32, tag="fidx")
            fidxs[t] = fidx
            nc.vector.tensor_copy(fidx[:], fin[:])  # f32 -> i32

        def w_gather(t):
            wout = wo.tile([P, OUT_DIM], F32, tag="wout")
            wouts[t] = wout
            nc.gpsimd.indirect_dma_start(
                out=wout[:],
                out_offset=None,
                in_=wt[:],
                in_offset=bass.IndirectOffsetOnAxis(ap=fidxs[t][:, :1], axis=0),
            )

        def store_t(t):
            nc.scalar.dma_start(y[t * P : (t + 1) * P, :], wouts[t][:])

        # Global emission order sets per-engine stream priorities:
        #   Vector: R0 R1 F0 R2 F1 ... R6 F5 R7a R7b F6 R7c pick7 F7
        #   GpSimd: CG0 CG1 WG0 CG2 WG1 ... CG6 WG5 WG6 CG7 WG7
        pick_t(0, reduce_t(0))
        chunk_gather(0)
        for t in range(1, T_LAST):
            pick_t(t, reduce_t(t))
            chunk_gather(t)
            find_t(t - 1)
            w_gather(t - 1)
            store_t(t - 1)
        # tile 7: reduce arrives in three pieces
        m7 = mp.tile([P, NCH], F32, tag="m")
        bounds = [0] + [c1 // CH for _, c1 in PIECES]
        nc.vector.reduce_max(
            m7[:, bounds[0] : bounds[1]],
            xt7[:, : PIECES[0][1]].rearrange("p (c e) -> p c e", e=CH),
            axis=mybir.AxisListType.X,
        )
        nc.vector.reduce_max(
            m7[:, bounds[1] : bounds[2]],
            xt7[:, PIECES[1][0] : PIECES[1][1]].rearrange(
                "p (c e) -> p c e", e=CH
            ),
            axis=mybir.AxisListType.X,
        )
        find_t(T_LAST - 1)
        nc.vector.reduce_max(
            m7[:, bounds[2] : bounds[3]],
            xt7[:, PIECES[2][0] : PIECES[2][1]].rearrange(
                "p (c e) -> p c e", e=CH
            ),
            axis=mybir.AxisListType.X,
        )
        pick_t(T_LAST, m7)
        w_gather(T_LAST - 1)
        store_t(T_LAST - 1)
        chunk_gather(T_LAST)
        find_t(T_LAST)
        w_gather(T_LAST)
        store_t(T_LAST)


_CACHE: dict[str, object] = {}


def _build():
    if "nc" in _CACHE:
        return _CACHE["nc"]
    nc = bacc.Bacc(
        "TRN2",
        target_bir_lowering=False,
        debug=False,
        enable_asserts=True,
        num_swdge_queues=1,
    )
    x = nc.dram_tensor("x", [ROWS, QUANT_DIM], F32, kind="ExternalInput").ap()
    wt = nc.dram_tensor("wt", [QUANT_DIM, OUT_DIM], F32, kind="ExternalInput").ap()
    y = nc.dram_tensor("y", [ROWS, OUT_DIM], F32, kind="ExternalOutput").ap()
    emit = {
        1: _emit_kernel,
        2: _emit_kernel_v2,
        3: _emit_kernel_v3,
        4: _emit_kernel_v4,
        5: _emit_kernel_v5,
        6: _emit_kernel_v6,
        7: _emit_kernel_v7,
        8: _emit_kernel_v8,
    }[VERSION]
    with tile.TileContext(nc) as tc:
        emit(tc, y, x, wt)
    nc.compile()
    _CACHE["nc"] = nc
    return nc


def kernel(x: np.ndarray, W: np.ndarray, **_unused) -> np.ndarray:
    assert x.shape == (N_TOKENS, QUANT_DIM) and W.shape == (OUT_DIM, QUANT_DIM)
    nc = _build()
    x = np.ascontiguousarray(x, dtype=np.float32)
    wt = np.ascontiguousarray(W.T.astype(np.float32, copy=False))
    in_maps = [
        {"x": x[i * ROWS : (i + 1) * ROWS], "wt": wt} for i in range(N_CORES)
    ]
    res = bass_utils.run_bass_kernel_spmd(nc, in_maps, core_ids=list(range(N_CORES)))
    return np.concatenate([res.results[i]["y"] for i in range(N_CORES)], axis=0)



# revision 8
# speedup vs baseline: 1.9068x; 1.3284x over previous
"""VQ codebook-lookup kernel for Trainium2 (8 NeuronCores, data-parallel).

Computes: idx = argmax(x, axis=-1); out = W.T[idx]  (i.e. out[n] = W[:, idx[n]])
  x: [8192, 8192] f32, W: [1024, 8192] f32 -> out: [8192, 1024] f32

Sharding: x split along tokens into 8 shards of [1024, 8192]; W.T replicated.
Each core: per-row argmax via VectorE max/max_index, then a DMA row-gather
from the transposed codebook in HBM.
"""

import sys

import numpy as np

sys.path.insert(0, "/opt/trn_rl_repo")

import concourse.bass as bass  # noqa: E402
import concourse.tile as tile  # noqa: E402
from concourse import bacc, bass_utils, mybir  # noqa: E402

N_CORES = 8
N_TOKENS = 8192
QUANT_DIM = 8192
OUT_DIM = 1024
ROWS = N_TOKENS // N_CORES  # rows (tokens) per core
P = 128  # SBUF partitions
N_TILES = ROWS // P  # row-tiles per core

F32 = mybir.dt.float32
I16 = mybir.dt.int16
I32 = mybir.dt.int32
U32 = mybir.dt.uint32

CH = 256  # chunk size for the hierarchical argmax
NCH = QUANT_DIM // CH  # 32 chunks per row

VERSION = 9
BATCHES = [(0, 4), (4, 3), (7, 1)]  # (first tile, n tiles) tail groups
WQ = 0  # single SWDGE queue (DMASW sem lanes are queue-locked; multi-queue trips Tile)


def _emit_kernel(tc: tile.TileContext, y: "bass.AP", x: "bass.AP", wt: "bass.AP"):
    """Per-core program. x: [ROWS, QUANT_DIM], wt: [QUANT_DIM, OUT_DIM] (=W.T),
    y: [ROWS, OUT_DIM]."""
    nc = tc.nc
    with (
        tc.tile_pool(name="xp", bufs=3) as xp,
        tc.tile_pool(name="sm", bufs=2 * N_TILES) as sm,
        tc.tile_pool(name="ip", bufs=1) as ip,
        tc.tile_pool(name="op", bufs=1) as op,
        tc.tile_pool(name="dr", bufs=1, space="DRAM") as dr,
    ):
        # Per-row argmax, one [128, QUANT_DIM] tile at a time.
        idx_all = ip.tile([P, N_TILES], I16)  # [row-in-tile, tile] argmax
        for t in range(N_TILES):
            xt = xp.tile([P, QUANT_DIM], F32)
            nc.sync.dma_start(xt[:], x[t * P : (t + 1) * P, :])
            mx = sm.tile([P, 8], F32, tag="mx")
            nc.vector.max(mx[:], xt[:])
            ix = sm.tile([P, 8], U32, tag="ix")
            nc.vector.max_index(ix[:], mx[:], xt[:])
            # argmax = ix[:, 0]; value < 8192 so the low half-word holds it.
            nc.vector.tensor_copy(idx_all[:, t : t + 1], ix.bitcast(I16)[:, 0:1])

        # dma_gather wants indices int16, "wrapped": gather j reads the index
        # at partition j%16, slot j//16 (replicated across the 8 gpsimd cores'
        # 16-partition groups). Row j = t*128 + p with p = s1*16 + q, so the
        # value for (q, slot=t*8+s1) is idx_all[s1*16+q, t]. Partition-crossing
        # shuffle goes through a DRAM scratch roundtrip.
        scratch = dr.tile([P, N_TILES], I16)
        nc.sync.dma_start(scratch[:], idx_all[:])
        idxw = ip.tile([P, ROWS // 16], I16)
        src = scratch.rearrange("(s1 q) t -> q t s1", q=16)
        for r in range(P // 16):
            dst = idxw[16 * r : 16 * (r + 1), :].rearrange(
                "q (t s1) -> q t s1", s1=N_TILES
            )
            nc.sync.dma_start(dst, src)

        # Gather rows of W.T from HBM: wout[p, t, :] = wt[idx[t*128+p], :]
        wout = op.tile([P, N_TILES, OUT_DIM], F32)
        nc.gpsimd.dma_gather(
            wout[:],
            wt[:],
            idxw[:],
            num_idxs=ROWS,
            num_idxs_reg=ROWS,
            elem_size=OUT_DIM,
        )
        nc.sync.dma_start(y.rearrange("(t p) d -> p t d", p=P), wout[:])


def _wrap_1024(nc, idx_sbuf, scratch, idxw):
    """Turn idx_sbuf [128, 8] int16 (value for row t*128+p at [p, t]) into the
    dma_gather wrapped layout idxw [128, 64]: gather j reads partition j%16,
    slot j//16; replicated across the 8 gpsimd-core partition groups.
    Row j = t*128 + s1*16 + q -> idxw[q, t*8+s1] = idx_sbuf[s1*16+q, t]."""
    nc.sync.dma_start(scratch[:], idx_sbuf[:])
    src = scratch.rearrange("(s1 q) t -> q t s1", q=16)
    for r in range(P // 16):
        dst = idxw[16 * r : 16 * (r + 1), :].rearrange(
            "q (t s1) -> q t s1", s1=N_TILES
        )
        nc.sync.dma_start(dst, src)


def _emit_kernel_v2(tc: tile.TileContext, y: "bass.AP", x: "bass.AP", wt: "bass.AP"):
    """Hierarchical argmax: one full pass computes per-chunk maxes (CH=256),
    cheap top-8 picks the winning chunk, a small HBM gather re-reads only the
    winning 1KB chunk per row, and a second tiny max/max_index finds the
    offset within it. ~1.06 passes of DVE work instead of 2."""
    nc = tc.nc
    with (
        tc.tile_pool(name="xp", bufs=3) as xp,
        tc.tile_pool(name="mp", bufs=N_TILES) as mp,
        tc.tile_pool(name="sm", bufs=2 * N_TILES) as sm,
        tc.tile_pool(name="keep", bufs=1) as keep,
        tc.tile_pool(name="op", bufs=1) as op,
        tc.tile_pool(name="dr", bufs=1, space="DRAM") as dr,
    ):
        # per-partition offsets: p*32 (global chunk id) as f32
        iota32 = keep.tile([P, 1], F32)
        nc.gpsimd.iota(
            iota32[:],
            pattern=[[0, 1]],
            base=0,
            channel_multiplier=NCH,
            allow_small_or_imprecise_dtypes=True,
        )

        cr_all = keep.tile([P, N_TILES], F32)  # winning chunk within row
        cidx_all = keep.tile([P, N_TILES], I16)  # global chunk id for gather
        # Phase A: chunk maxes + winning chunk per row.
        for t in range(N_TILES):
            xt = xp.tile([P, QUANT_DIM], F32)
            nc.sync.dma_start(xt[:], x[t * P : (t + 1) * P, :])
            m = mp.tile([P, NCH], F32, tag="m")
            nc.vector.reduce_max(
                m[:], xt.rearrange("p (c e) -> p c e", e=CH), axis=mybir.AxisListType.X
            )
            mx8 = sm.tile([P, 8], F32, tag="mx8")
            nc.vector.max(mx8[:], m[:])
            ci8 = sm.tile([P, 8], U32, tag="ci8")
            nc.vector.max_index(ci8[:], mx8[:], m[:])
            nc.vector.tensor_copy(cr_all[:, t : t + 1], ci8[:, 0:1])  # u32->f32
            gci = sm.tile([P, 1], F32, tag="gci")
            # global chunk id = (cr + t*128*NCH) + p*NCH
            nc.vector.tensor_scalar(
                gci[:],
                cr_all[:, t : t + 1],
                float(t * P * NCH),
                iota32[:],
                op0=mybir.AluOpType.add,
                op1=mybir.AluOpType.add,
            )
            nc.vector.tensor_copy(cidx_all[:, t : t + 1], gci[:])  # f32->i16

        # Phase B: gather each row's winning chunk (1KB) from x in HBM.
        scr_c = dr.tile([P, N_TILES], I16, tag="scr_c")
        idxw_c = keep.tile([P, ROWS // 16], I16, tag="idxw_c")
        _wrap_1024(nc, cidx_all, scr_c, idxw_c)
        wch = keep.tile([P, N_TILES, CH], F32)
        nc.gpsimd.dma_gather(
            wch[:],
            x.rearrange("r (c e) -> (r c) e", e=CH),
            idxw_c[:],
            num_idxs=ROWS,
            num_idxs_reg=ROWS,
            elem_size=CH,
        )

        # Phase C: offset within the winning chunk; final row-argmax.
        idx_all = keep.tile([P, N_TILES], I16)
        for t in range(N_TILES):
            wmx8 = sm.tile([P, 8], F32, tag="wmx8")
            nc.vector.max(wmx8[:], wch[:, t, :])
            wix8 = sm.tile([P, 8], U32, tag="wix8")
            nc.vector.max_index(wix8[:], wmx8[:], wch[:, t, :])
            wif = sm.tile([P, 1], F32, tag="wif")
            nc.vector.tensor_copy(wif[:], wix8[:, 0:1])  # u32->f32
            fin = sm.tile([P, 1], F32, tag="fin")
            # final = cr*CH + wi
            nc.vector.tensor_scalar(
                fin[:],
                cr_all[:, t : t + 1],
                float(CH),
                wif[:],
                op0=mybir.AluOpType.mult,
                op1=mybir.AluOpType.add,
            )
            nc.vector.tensor_copy(idx_all[:, t : t + 1], fin[:])  # f32->i16

        # Phase D: gather rows of W.T and write out.
        scr_w = dr.tile([P, N_TILES], I16, tag="scr_w")
        idxw_w = keep.tile([P, ROWS // 16], I16, tag="idxw_w")
        _wrap_1024(nc, idx_all, scr_w, idxw_w)
        wout = op.tile([P, N_TILES, OUT_DIM], F32)
        nc.gpsimd.dma_gather(
            wout[:],
            wt[:],
            idxw_w[:],
            num_idxs=ROWS,
            num_idxs_reg=ROWS,
            elem_size=OUT_DIM,
        )
        nc.sync.dma_start(y.rearrange("(t p) d -> p t d", p=P), wout[:])


def _emit_kernel_v3(tc: tile.TileContext, y: "bass.AP", x: "bass.AP", wt: "bass.AP"):
    """Fully per-tile pipelined hierarchical argmax + gather.

    Each [128, 8192] row-tile runs its complete chain (chunk-max reduce ->
    winning chunk -> 1KB/row chunk re-gather -> within-chunk argmax -> W.T row
    gather -> output store) independently, so the chains of tiles 0..6 hide
    under the HBM-bound loads of later tiles; only the last tile's ~20us chain
    sits on the critical path. Index wraps (partition redistribution into the
    dma_gather layout: idx j at partition j%16, slot j//16, replicated into
    partitions 16..31 for the second Q7 core of the queue) go through a DRAM
    scratch roundtrip issued on the otherwise-idle Scalar/Tensor sequencers.
    """
    nc = tc.nc
    with (
        tc.tile_pool(name="xp", bufs=3) as xp,
        tc.tile_pool(name="mp", bufs=3) as mp,
        tc.tile_pool(name="sm", bufs=3) as sm,
        tc.tile_pool(name="iw", bufs=3) as iw,
        tc.tile_pool(name="wc", bufs=3) as wc,
        tc.tile_pool(name="wo", bufs=3) as wo,
        tc.tile_pool(name="keep", bufs=1) as keep,
        tc.tile_pool(name="dr", bufs=3, space="DRAM") as dr,
    ):
        iota32 = keep.tile([P, 1], F32)  # p*NCH per partition
        nc.gpsimd.iota(
            iota32[:],
            pattern=[[0, 1]],
            base=0,
            channel_multiplier=NCH,
            allow_small_or_imprecise_dtypes=True,
        )

        for t in range(N_TILES):
            # ---- load + chunk maxes ----
            xt = xp.tile([P, QUANT_DIM], F32, tag="xt")
            nc.sync.dma_start(xt[:], x[t * P : (t + 1) * P, :])
            m = mp.tile([P, NCH], F32, tag="m")
            nc.vector.reduce_max(
                m[:], xt.rearrange("p (c e) -> p c e", e=CH), axis=mybir.AxisListType.X
            )
            mx8 = sm.tile([P, 8], F32, tag="mx8")
            nc.vector.max(mx8[:], m[:])
            ci8 = sm.tile([P, 8], U32, tag="ci8")
            nc.vector.max_index(ci8[:], mx8[:], m[:])
            crf = sm.tile([P, 1], F32, tag="crf")
            nc.vector.tensor_copy(crf[:], ci8[:, 0:1])  # u32 -> f32
            # chunk id within this tile's 4096 chunks: p*NCH + cr
            gci = sm.tile([P, 1], F32, tag="gci")
            nc.vector.tensor_scalar_add(gci[:], crf[:], iota32[:])
            gci16 = sm.tile([P, 1], I16, tag="gci16")
            nc.vector.tensor_copy(gci16[:], gci[:])  # f32 -> i16

            # ---- wrap chunk idx + 1KB/row chunk re-gather ----
            scr_c = dr.tile([P, 1], I16, tag="scr_c")
            nc.scalar.dma_start(scr_c[:], gci16[:])
            idxw_c = iw.tile([P, N_TILES], I16, tag="idxw_c")
            nc.gpsimd.memset(idxw_c[:], 0)
            src_c = scr_c.rearrange("(s1 q) one -> q (s1 one)", q=16)
            nc.scalar.dma_start(idxw_c[0:16, :], src_c)
            nc.scalar.dma_start(idxw_c[16:32, :], src_c)
            wch = wc.tile([P, 1, CH], F32, tag="wch")
            nc.gpsimd.dma_gather(
                wch[:],
                x[t * P : (t + 1) * P, :].rearrange("p (c e) -> (p c) e", e=CH),
                idxw_c[:],
                num_idxs=P,
                num_idxs_reg=P,
                elem_size=CH,
            )

            # ---- within-chunk offset; final row argmax ----
            wix8 = sm.tile([P, 8], U32, tag="wix8")
            nc.vector.max_index(wix8[:], mx8[:], wch[:, 0, :])
            wif = sm.tile([P, 1], F32, tag="wif")
            nc.vector.tensor_copy(wif[:], wix8[:, 0:1])  # u32 -> f32
            fin = sm.tile([P, 1], F32, tag="fin")
            nc.vector.tensor_scalar(
                fin[:],
                crf[:],
                float(CH),
                wif[:],
                op0=mybir.AluOpType.mult,
                op1=mybir.AluOpType.add,
            )
            fin16 = sm.tile([P, 1], I16, tag="fin16")
            nc.vector.tensor_copy(fin16[:], fin[:])  # f32 -> i16

            # ---- wrap final idx + gather W.T rows + store ----
            scr_w = dr.tile([P, 1], I16, tag="scr_w")
            nc.scalar.dma_start(scr_w[:], fin16[:])
            idxw_w = iw.tile([P, N_TILES], I16, tag="idxw_w")
            nc.gpsimd.memset(idxw_w[:], 0)
            src_w = scr_w.rearrange("(s1 q) one -> q (s1 one)", q=16)
            nc.scalar.dma_start(idxw_w[0:16, :], src_w)
            nc.scalar.dma_start(idxw_w[16:32, :], src_w)
            wout = wo.tile([P, 1, OUT_DIM], F32, tag="wout")
            nc.gpsimd.dma_gather(
                wout[:],
                wt[:],
                idxw_w[:],
                num_idxs=P,
                num_idxs_reg=P,
                elem_size=OUT_DIM,
            )
            nc.sync.dma_start(y[t * P : (t + 1) * P, :], wout[:, 0, :])


def _emit_kernel_v4(tc: tile.TileContext, y: "bass.AP", x: "bass.AP", wt: "bass.AP"):
    """Software-pipelined hierarchical argmax + gather.

    Same per-tile dataflow as v3, but emitted stage-major so each engine's
    in-order instruction stream never head-of-line blocks: all 8 HBM loads
    queue first on the SP HWDGE ring, the per-tile chains are interleaved with
    a 1-tile stagger (tile t's within-chunk stage emitted after tile t+1's
    chunk stage), and the output stores queue last.
    """
    nc = tc.nc
    with (
        tc.tile_pool(name="xp", bufs=4) as xp,
        tc.tile_pool(name="mp", bufs=3) as mp,
        tc.tile_pool(name="sm", bufs=3) as sm,
        tc.tile_pool(name="iw", bufs=3) as iw,
        tc.tile_pool(name="wc", bufs=3) as wc,
        tc.tile_pool(name="wo", bufs=8) as wo,
        tc.tile_pool(name="keep", bufs=1) as keep,
        tc.tile_pool(name="dr", bufs=3, space="DRAM") as dr,
    ):
        iota32 = keep.tile([P, 1], F32)  # p*NCH per partition
        nc.gpsimd.iota(
            iota32[:],
            pattern=[[0, 1]],
            base=0,
            channel_multiplier=NCH,
            allow_small_or_imprecise_dtypes=True,
        )

        # Stage 0: queue every HBM load up front (SP ring stays saturated).
        xts = []
        for t in range(N_TILES):
            xt = xp.tile([P, QUANT_DIM], F32, tag="xt")
            nc.sync.dma_start(xt[:], x[t * P : (t + 1) * P, :])
            xts.append(xt)

        crfs = [None] * N_TILES
        mx8s = [None] * N_TILES
        wchs = [None] * N_TILES
        wouts = [None] * N_TILES

        def stage_a(t):
            """chunk maxes -> winning chunk -> wrap -> 1KB/row chunk gather"""
            m = mp.tile([P, NCH], F32, tag="m")
            nc.vector.reduce_max(
                m[:],
                xts[t].rearrange("p (c e) -> p c e", e=CH),
                axis=mybir.AxisListType.X,
            )
            mx8 = sm.tile([P, 8], F32, tag="mx8")
            mx8s[t] = mx8
            nc.vector.max(mx8[:], m[:])
            ci8 = sm.tile([P, 8], U32, tag="ci8")
            nc.vector.max_index(ci8[:], mx8[:], m[:])
            crf = sm.tile([P, 1], F32, tag="crf")
            crfs[t] = crf
            nc.vector.tensor_copy(crf[:], ci8[:, 0:1])  # u32 -> f32
            gci = sm.tile([P, 1], F32, tag="gci")
            nc.vector.tensor_scalar_add(gci[:], crf[:], iota32[:])
            gci16 = sm.tile([P, 1], I16, tag="gci16")
            nc.vector.tensor_copy(gci16[:], gci[:])  # f32 -> i16

            scr_c = dr.tile([P, 1], I16, tag="scr_c")
            nc.scalar.dma_start(scr_c[:], gci16[:])
            idxw_c = iw.tile([P, N_TILES], I16, tag="idxw_c")
            nc.gpsimd.memset(idxw_c[:], 0)
            src_c = scr_c.rearrange("(s1 q) one -> q (s1 one)", q=16)
            nc.scalar.dma_start(idxw_c[0:16, :], src_c)
            nc.scalar.dma_start(idxw_c[16:32, :], src_c)
            wch = wc.tile([P, 1, CH], F32, tag="wch")
            wchs[t] = wch
            nc.gpsimd.dma_gather(
                wch[:],
                x[t * P : (t + 1) * P, :].rearrange("p (c e) -> (p c) e", e=CH),
                idxw_c[:],
                num_idxs=P,
                num_idxs_reg=P,
                elem_size=CH,
            )

        def stage_b(t):
            """within-chunk offset -> final idx -> wrap -> W.T row gather"""
            wix8 = sm.tile([P, 8], U32, tag="wix8")
            nc.vector.max_index(wix8[:], mx8s[t][:], wchs[t][:, 0, :])
            wif = sm.tile([P, 1], F32, tag="wif")
            nc.vector.tensor_copy(wif[:], wix8[:, 0:1])  # u32 -> f32
            fin = sm.tile([P, 1], F32, tag="fin")
            nc.vector.tensor_scalar(
                fin[:],
                crfs[t][:],
                float(CH),
                wif[:],
                op0=mybir.AluOpType.mult,
                op1=mybir.AluOpType.add,
            )
            fin16 = sm.tile([P, 1], I16, tag="fin16")
            nc.vector.tensor_copy(fin16[:], fin[:])  # f32 -> i16

            scr_w = dr.tile([P, 1], I16, tag="scr_w")
            nc.scalar.dma_start(scr_w[:], fin16[:])
            idxw_w = iw.tile([P, N_TILES], I16, tag="idxw_w")
            nc.gpsimd.memset(idxw_w[:], 0)
            src_w = scr_w.rearrange("(s1 q) one -> q (s1 one)", q=16)
            nc.scalar.dma_start(idxw_w[0:16, :], src_w)
            nc.scalar.dma_start(idxw_w[16:32, :], src_w)
            wout = wo.tile([P, 1, OUT_DIM], F32, tag="wout")
            wouts[t] = wout
            nc.gpsimd.dma_gather(
                wout[:],
                wt[:],
                idxw_w[:],
                num_idxs=P,
                num_idxs_reg=P,
                elem_size=OUT_DIM,
            )

        # 1-tile stagger: ... a(t), b(t-1), a(t+1), b(t) ...
        stage_a(0)
        for t in range(1, N_TILES):
            stage_a(t)
            stage_b(t - 1)
        stage_b(N_TILES - 1)

        # Stage Z: output stores, queued after the loads on the SP ring.
        for t in range(N_TILES):
            nc.sync.dma_start(y[t * P : (t + 1) * P, :], wouts[t][:, 0, :])


def _emit_kernel_v5(tc: tile.TileContext, y: "bass.AP", x: "bass.AP", wt: "bass.AP"):
    """Batched stage-major pipeline.

    All 8 HBM loads queue first and stream at full bandwidth; the per-row
    reduce/pick runs behind each load. Tail stages (index wrap -> chunk
    re-gather -> within-chunk argmax -> W.T gather -> store) run per BATCH of
    tiles: the first batch's tail hides under the second batch's loads, so only
    the last batch's ~30us tail sits on the critical path. Few, coarse DMAs
    keep the shared DMA-completion semaphore lanes from creating false
    cross-dependencies (which serialized the fine-grained variant).
    A dummy 16-row gather up front pre-loads the Q7 dma_gather ucode.
    """
    nc = tc.nc
    nb = len(BATCHES)
    with (
        tc.tile_pool(name="xp", bufs=4) as xp,
        tc.tile_pool(name="mp", bufs=3) as mp,
        tc.tile_pool(name="sm", bufs=4) as sm,
        tc.tile_pool(name="pk", bufs=N_TILES) as pk,
        tc.tile_pool(name="bt", bufs=2) as bt,
        tc.tile_pool(name="wc", bufs=2) as wc,
        tc.tile_pool(name="wo", bufs=2) as wo,
        tc.tile_pool(name="keep", bufs=1) as keep,
        tc.tile_pool(name="dr", bufs=2, space="DRAM") as dr,
    ):
        # Warm the Q7 dma_gather ucode while the first loads stream.
        widx = keep.tile([P, 1], I16)
        nc.gpsimd.memset(widx[:], 0)
        wscrap = keep.tile([P, 1, 64], F32)
        nc.gpsimd.dma_gather(
            wscrap[:],
            wt[:, 0:64],
            widx[:],
            num_idxs=16,
            num_idxs_reg=16,
            elem_size=64,
            elem_step=OUT_DIM,
        )

        # Stage 0: queue every HBM load up front.
        xts = []
        for t in range(N_TILES):
            xt = xp.tile([P, QUANT_DIM], F32, tag="xt")
            nc.sync.dma_start(xt[:], x[t * P : (t + 1) * P, :])
            xts.append(xt)

        iota32 = keep.tile([P, 1], F32)  # p*NCH per partition
        nc.gpsimd.iota(
            iota32[:],
            pattern=[[0, 1]],
            base=0,
            channel_multiplier=NCH,
            allow_small_or_imprecise_dtypes=True,
        )

        mx8s = [None] * N_TILES
        crfs = [None] * N_TILES

        def reduce_pick(t, i, cidx_b):
            """chunk maxes + winning chunk for tile t (column i of the batch)"""
            m = mp.tile([P, NCH], F32, tag="m")
            nc.vector.reduce_max(
                m[:],
                xts[t].rearrange("p (c e) -> p c e", e=CH),
                axis=mybir.AxisListType.X,
            )
            mx8 = pk.tile([P, 8], F32, tag="mx8")
            mx8s[t] = mx8
            nc.vector.max(mx8[:], m[:])
            ci8 = sm.tile([P, 8], U32, tag="ci8")
            nc.vector.max_index(ci8[:], mx8[:], m[:])
            crf = pk.tile([P, 1], F32, tag="crf")
            crfs[t] = crf
            nc.vector.tensor_copy(crf[:], ci8[:, 0:1])  # u32 -> f32
            gci = sm.tile([P, 1], F32, tag="gci")
            # chunk id within the batch's gather space: i*128*NCH + p*NCH + cr
            nc.vector.tensor_scalar(
                gci[:],
                crf[:],
                float(i * P * NCH),
                iota32[:],
                op0=mybir.AluOpType.add,
                op1=mybir.AluOpType.add,
            )
            nc.vector.tensor_copy(cidx_b[:, i : i + 1], gci[:])  # f32 -> i16
            return red

        def wrap(idx_b, n, scr_tag, idxw_tag, engine):
            """[128, n] i16 (value for row i*128+p at [p, i]) -> wrapped
            [128, 8n]: gather j reads partition j%16, slot j//16; replicated to
            partitions 16..31 for the queue's second Q7 core."""
            scr = dr.tile([P, n], I16, tag=scr_tag)
            engine.dma_start(scr[:], idx_b[:])
            idxw = bt.tile([P, 8 * n], I16, tag=idxw_tag)
            nc.gpsimd.memset(idxw[:], 0)
            src = scr.rearrange("(s1 q) i -> q i s1", q=16)
            dst0 = idxw[0:16, :].rearrange("q (i s1) -> q i s1", s1=8)
            dst1 = idxw[16:32, :].rearrange("q (i s1) -> q i s1", s1=8)
            engine.dma_start(dst0, src)
            engine.dma_start(dst1, src)
            return idxw

        def chunk_stage(b0, n, cidx_b):
            idxw_c = wrap(cidx_b, n, "scr_c", "idxw_c", nc.scalar)
            wch = wc.tile([P, n, CH], F32, tag="wch")
            nc.gpsimd.dma_gather(
                wch[:],
                x[b0 * P : (b0 + n) * P, :].rearrange("r (c e) -> (r c) e", e=CH),
                idxw_c[:],
                num_idxs=n * P,
                num_idxs_reg=n * P,
                elem_size=CH,
            )
            return wch

        def within(t, i, wch, fidx_b):
            """within-chunk offset -> final row argmax (column i of batch)"""
            wix8 = sm.tile([P, 8], U32, tag="wix8")
            nc.vector.max_index(wix8[:], mx8s[t][:], wch[:, i, :])
            wif = sm.tile([P, 1], F32, tag="wif")
            nc.vector.tensor_copy(wif[:], wix8[:, 0:1])  # u32 -> f32
            fin = sm.tile([P, 1], F32, tag="fin")
            nc.vector.tensor_scalar(
                fin[:],
                crfs[t][:],
                float(CH),
                wif[:],
                op0=mybir.AluOpType.mult,
                op1=mybir.AluOpType.add,
            )
            nc.vector.tensor_copy(fidx_b[:, i : i + 1], fin[:])  # f32 -> i16

        def out_stage(b0, n, fidx_b):
            idxw_w = wrap(fidx_b, n, "scr_w", "idxw_w", nc.scalar)
            wout = wo.tile([P, n, OUT_DIM], F32, tag="wout")
            nc.gpsimd.dma_gather(
                wout[:],
                wt[:],
                idxw_w[:],
                num_idxs=n * P,
                num_idxs_reg=n * P,
                elem_size=OUT_DIM,
            )
            nc.sync.dma_start(
                y[b0 * P : (b0 + n) * P, :].rearrange("(i p) d -> p i d", p=P),
                wout[:],
            )

        # Interleave: batch b's tail stages are emitted just after the first
        # reduce of batch b+1, so they hide under the remaining loads.
        pending = None  # (b0, n, cidx_b, wch-to-come...)
        for bi, (b0, n) in enumerate(BATCHES):
            cidx_b = bt.tile([P, n], I16, tag="cidx")
            for k in range(n):
                reduce_pick(b0 + k, k, cidx_b)
                if k == 0 and pending is not None:
                    pb0, pn, pcidx = pending
                    wch = chunk_stage(pb0, pn, pcidx)
                    fidx_b = bt.tile([P, pn], I16, tag="fidx")
                    for j in range(pn):
                        within(pb0 + j, j, wch, fidx_b)
                    out_stage(pb0, pn, fidx_b)
                    pending = None
            pending = (b0, n, cidx_b)

        pb0, pn, pcidx = pending
        wch = chunk_stage(pb0, pn, pcidx)
        fidx_b = bt.tile([P, pn], I16, tag="fidx")
        for j in range(pn):
            within(pb0 + j, j, wch, fidx_b)
        out_stage(pb0, pn, fidx_b)


def _emit_kernel_v6(tc: tile.TileContext, y: "bass.AP", x: "bass.AP", wt: "bass.AP"):
    """v5 + three fixes that came out of the v5 trace:

    - Index-wrap DMAs ride SWDGE (gpsimd.dma_start) instead of HWDGE: the 8
      HWDGE completion-semaphore lanes are shared round-robin, so a tiny wrap
      read could end up waiting on a still-running 4MB x load (observed ~20us
      false stalls).  SWDGE has its own lanes.
    - The W-row gathers run on SWDGE queue 1 (own Q7 core pair + ring), so
      their multi-MB transfers never head-of-line block the next batch's wrap
      writes/chunk gather on queue 0.  Queue 1's cores read the wrapped index
      buffer from partitions 32..63, CoreSim reads 0..15 - replicate to both.
    - The tail batches shrink (4/3/1) so the final batch's chain is minimal.
    """
    nc = tc.nc
    with (
        tc.tile_pool(name="xp", bufs=5) as xp,
        tc.tile_pool(name="mp", bufs=3) as mp,
        tc.tile_pool(name="sm", bufs=4) as sm,
        tc.tile_pool(name="pk", bufs=N_TILES) as pk,
        tc.tile_pool(name="bt", bufs=2) as bt,
        tc.tile_pool(name="wc", bufs=2) as wc,
        tc.tile_pool(name="wo", bufs=1) as wo,
        tc.tile_pool(name="keep", bufs=1) as keep,
        tc.tile_pool(name="dr", bufs=2, space="DRAM") as dr,
    ):
        # Warm the Q7 dma_gather ucode on both queues while loads stream.
        widx = keep.tile([P, 1], I16)
        nc.gpsimd.memset(widx[:], 0)
        for q in (0, WQ):
            wscrap = keep.tile([P, 1, 64], F32, tag=f"wscrap{q}")
            nc.gpsimd.dma_gather(
                wscrap[:],
                wt[:, 0:64],
                widx[:],
                num_idxs=16,
                num_idxs_reg=16,
                elem_size=64,
                elem_step=OUT_DIM,
                queue_num=q,
            )

        # Queue every HBM load up front.
        xts = []
        for t in range(N_TILES):
            xt = xp.tile([P, QUANT_DIM], F32, tag="xt")
            nc.sync.dma_start(xt[:], x[t * P : (t + 1) * P, :])
            xts.append(xt)

        iota32 = keep.tile([P, 1], F32)  # p*NCH per partition
        nc.gpsimd.iota(
            iota32[:],
            pattern=[[0, 1]],
            base=0,
            channel_multiplier=NCH,
            allow_small_or_imprecise_dtypes=True,
        )

        mx8s = [None] * N_TILES
        crfs = [None] * N_TILES

        def reduce_pick(t, i, cidx_b):
            m = mp.tile([P, NCH], F32, tag="m")
            red = nc.vector.reduce_max(
                m[:],
                xts[t].rearrange("p (c e) -> p c e", e=CH),
                axis=mybir.AxisListType.X,
            )
            mx8 = pk.tile([P, 8], F32, tag="mx8")
            mx8s[t] = mx8
            nc.vector.max(mx8[:], m[:])
            ci8 = sm.tile([P, 8], U32, tag="ci8")
            nc.vector.max_index(ci8[:], mx8[:], m[:])
            crf = pk.tile([P, 1], F32, tag="crf")
            crfs[t] = crf
            nc.vector.tensor_copy(crf[:], ci8[:, 0:1])  # u32 -> f32
            gci = sm.tile([P, 1], F32, tag="gci")
            nc.vector.tensor_scalar(
                gci[:],
                crf[:],
                float(i * P * NCH),
                iota32[:],
                op0=mybir.AluOpType.add,
                op1=mybir.AluOpType.add,
            )
            nc.vector.tensor_copy(cidx_b[:, i : i + 1], gci[:])  # f32 -> i16
            return red

        def wrap(idx_b, n, scr_tag, idxw_tag, groups, eng=None):
            """[128, n] i16 -> wrapped [128, 8n] via a DRAM roundtrip.
            Default engine is SWDGE (own completion-sem lanes, no false deps on
            in-flight HWDGE loads); the last batch uses scalar HWDGE (loads are
            done by then) to stay off the SWDGE ring behind big gathers.
            `groups` = 16-partition groups to fill (Q7 cores that will read)."""
            eng = eng or nc.gpsimd
            scr = dr.tile([P, n], I16, tag=scr_tag)
            eng.dma_start(scr[:], idx_b[:])
            idxw = bt.tile([P, 8 * n], I16, tag=idxw_tag)
            nc.gpsimd.memset(idxw[:], 0)
            src = scr.rearrange("(s1 q) i -> q i s1", q=16)
            for r in groups:
                dst = idxw[16 * r : 16 * (r + 1), :].rearrange(
                    "q (i s1) -> q i s1", s1=8
                )
                eng.dma_start(dst, src)
            return idxw

        def chunk_stage(b0, n, cidx_b, eng=None):
            idxw_c = wrap(cidx_b, n, "scr_c", "idxw_c", (0, 1), eng)
            wch = wc.tile([P, n, CH], F32, tag="wch")
            nc.gpsimd.dma_gather(
                wch[:],
                x[b0 * P : (b0 + n) * P, :].rearrange("r (c e) -> (r c) e", e=CH),
                idxw_c[:],
                num_idxs=n * P,
                num_idxs_reg=n * P,
                elem_size=CH,
            )
            return wch

        def within(t, i, wch, fidx_b, after=None):
            wix8 = sm.tile([P, 8], U32, tag="wix8")
            wix = nc.vector.max_index(wix8[:], mx8s[t][:], wch[:, i, :])
            if after is not None:
                # Keep this off the Vector stream until `after` has issued: the
                # scheduler's cost model underestimates the wrap+gather latency
                # and would otherwise park the stream here, stalling the
                # remaining reduces behind it (~25us on HW).
                tile.add_dep_helper(
                    wix.ins, after.ins, sync=False, reason="hold within behind reduce"
                )
            wif = sm.tile([P, 1], F32, tag="wif")
            nc.vector.tensor_copy(wif[:], wix8[:, 0:1])  # u32 -> f32
            fin = sm.tile([P, 1], F32, tag="fin")
            nc.vector.tensor_scalar(
                fin[:],
                crfs[t][:],
                float(CH),
                wif[:],
                op0=mybir.AluOpType.mult,
                op1=mybir.AluOpType.add,
            )
            nc.vector.tensor_copy(fidx_b[:, i : i + 1], fin[:])  # f32 -> i16

        def out_stage(b0, n, fidx_b, eng=None):
            wgroups = (0, 1) if WQ == 0 else (0, 2 * WQ, 2 * WQ + 1)
            idxw_w = wrap(fidx_b, n, "scr_w", "idxw_w", wgroups, eng)
            wout = wo.tile([P, n, OUT_DIM], F32, tag="wout")
            nc.gpsimd.dma_gather(
                wout[:],
                wt[:],
                idxw_w[:],
                num_idxs=n * P,
                num_idxs_reg=n * P,
                elem_size=OUT_DIM,
                queue_num=WQ,
            )
            nc.sync.dma_start(
                y[b0 * P : (b0 + n) * P, :].rearrange("(i p) d -> p i d", p=P),
                wout[:],
            )

        def emit_tail(b0, n, cidx_b, last=False):
            eng = nc.scalar if last else None
            wch = chunk_stage(b0, n, cidx_b, eng)
            fidx_b = bt.tile([P, n], I16, tag="fidx")
            for j in range(n):
                within(b0 + j, j, wch, fidx_b)
            out_stage(b0, n, fidx_b, eng)

        pending = None
        for b0, n in BATCHES:
            cidx_b = bt.tile([P, n], I16, tag="cidx")
            for k in range(n):
                reduce_pick(b0 + k, k, cidx_b)
                if k == 0 and pending is not None:
                    emit_tail(*pending)
                    pending = None
            pending = (b0, n, cidx_b)
        emit_tail(*pending, last=True)



def _emit_kernel_v7(tc: tile.TileContext, y: "bass.AP", x: "bass.AP", wt: "bass.AP"):
    """v6 helpers with a hand-scheduled emission for batches (4, 3, 1).

    Engine-stream plan (the Tile scheduler follows emission priority, so each
    engine's in-order stream must never park on a wait while later-ready work
    sits behind it):
      Vector: r0..r4 | r5 | within(b0) | r6 | r7 | within(b1) | within(b2)
      GpSimd: warmup, wrapC(b0), gatherC(b0), wrapW(b0), gatherW(b0),
              gatherC(b1), gatherC(b2), gatherW(b1), gatherW(b2)
      Scalar: wrapC(b1), wrapC(b2), wrapW(b1), wrapW(b2)   (HWDGE; loads are
              nearly drained by then so lane false-deps cost little)
      Sync:   loads 0..7, y(b0), y(b1), y(b2)
    b0's wraps ride SWDGE (loads still in flight -> HWDGE lanes unsafe); its
    2MB W-gather transfer finishes on the queue-0 ring before the later small
    wrap writes would need it, and the b1/b2 wraps avoid that ring entirely.
    """
    nc = tc.nc
    assert BATCHES == [(0, 4), (4, 3), (7, 1)]
    with (
        tc.tile_pool(name="xp", bufs=5) as xp,
        tc.tile_pool(name="mp", bufs=3) as mp,
        tc.tile_pool(name="sm", bufs=4) as sm,
        tc.tile_pool(name="pk", bufs=N_TILES) as pk,
        tc.tile_pool(name="bt", bufs=2) as bt,
        tc.tile_pool(name="wc", bufs=2) as wc,
        tc.tile_pool(name="wo", bufs=1) as wo,
        tc.tile_pool(name="keep", bufs=1) as keep,
        tc.tile_pool(name="dr", bufs=2, space="DRAM") as dr,
    ):
        # Warm the Q7 dma_gather ucode while the first loads stream.
        widx = keep.tile([P, 1], I16)
        nc.gpsimd.memset(widx[:], 0)
        wscrap = keep.tile([P, 1, 64], F32)
        nc.gpsimd.dma_gather(
            wscrap[:],
            wt[:, 0:64],
            widx[:],
            num_idxs=16,
            num_idxs_reg=16,
            elem_size=64,
            elem_step=OUT_DIM,
        )

        xts = []
        for t in range(N_TILES):
            xt = xp.tile([P, QUANT_DIM], F32, tag="xt")
            nc.sync.dma_start(xt[:], x[t * P : (t + 1) * P, :])
            xts.append(xt)

        iota32 = keep.tile([P, 1], F32)  # p*NCH per partition
        nc.gpsimd.iota(
            iota32[:],
            pattern=[[0, 1]],
            base=0,
            channel_multiplier=NCH,
            allow_small_or_imprecise_dtypes=True,
        )

        mx8s = [None] * N_TILES
        crfs = [None] * N_TILES

        def reduce_pick(t, i, cidx_b):
            m = mp.tile([P, NCH], F32, tag="m")
            red = nc.vector.reduce_max(
                m[:],
                xts[t].rearrange("p (c e) -> p c e", e=CH),
                axis=mybir.AxisListType.X,
            )
            mx8 = pk.tile([P, 8], F32, tag="mx8")
            mx8s[t] = mx8
            nc.vector.max(mx8[:], m[:])
            ci8 = sm.tile([P, 8], U32, tag="ci8")
            nc.vector.max_index(ci8[:], mx8[:], m[:])
            crf = pk.tile([P, 1], F32, tag="crf")
            crfs[t] = crf
            nc.vector.tensor_copy(crf[:], ci8[:, 0:1])  # u32 -> f32
            gci = sm.tile([P, 1], F32, tag="gci")
            nc.vector.tensor_scalar(
                gci[:],
                crf[:],
                float(i * P * NCH),
                iota32[:],
                op0=mybir.AluOpType.add,
                op1=mybir.AluOpType.add,
            )
            nc.vector.tensor_copy(cidx_b[:, i : i + 1], gci[:])  # f32 -> i16
            return red

        def wrap(idx_b, n, scr_tag, idxw_tag, eng):
            scr = dr.tile([P, n], I16, tag=scr_tag)
            eng.dma_start(scr[:], idx_b[:])
            idxw = bt.tile([P, 8 * n], I16, tag=idxw_tag)
            nc.gpsimd.memset(idxw[:], 0)
            src = scr.rearrange("(s1 q) i -> q i s1", q=16)
            for r in (0, 1):
                dst = idxw[16 * r : 16 * (r + 1), :].rearrange(
                    "q (i s1) -> q i s1", s1=8
                )
                eng.dma_start(dst, src)
            return idxw

        def gather_c(b0, n, idxw_c):
            wch = wc.tile([P, n, CH], F32, tag="wch")
            nc.gpsimd.dma_gather(
                wch[:],
                x[b0 * P : (b0 + n) * P, :].rearrange("r (c e) -> (r c) e", e=CH),
                idxw_c[:],
                num_idxs=n * P,
                num_idxs_reg=n * P,
                elem_size=CH,
            )
            return wch

        def within(t, i, wch, fidx_b, after=None):
            wix8 = sm.tile([P, 8], U32, tag="wix8")
            wix = nc.vector.max_index(wix8[:], mx8s[t][:], wch[:, i, :])
            if after is not None:
                # Keep this off the Vector stream until `after` has issued: the
                # scheduler's cost model underestimates the wrap+gather latency
                # and would otherwise park the stream here, stalling the
                # remaining reduces behind it (~25us on HW).
                tile.add_dep_helper(
                    wix.ins, after.ins, sync=False, reason="hold within behind reduce"
                )
            wif = sm.tile([P, 1], F32, tag="wif")
            nc.vector.tensor_copy(wif[:], wix8[:, 0:1])  # u32 -> f32
            fin = sm.tile([P, 1], F32, tag="fin")
            nc.vector.tensor_scalar(
                fin[:],
                crfs[t][:],
                float(CH),
                wif[:],
                op0=mybir.AluOpType.mult,
                op1=mybir.AluOpType.add,
            )
            nc.vector.tensor_copy(fidx_b[:, i : i + 1], fin[:])  # f32 -> i16

        def gather_w(n, idxw_w):
            wout = wo.tile([P, n, OUT_DIM], F32, tag="wout")
            nc.gpsimd.dma_gather(
                wout[:],
                wt[:],
                idxw_w[:],
                num_idxs=n * P,
                num_idxs_reg=n * P,
                elem_size=OUT_DIM,
            )
            return wout

        def store(b0, n, wout):
            nc.sync.dma_start(
                y[b0 * P : (b0 + n) * P, :].rearrange("(i p) d -> p i d", p=P),
                wout[:],
            )

        cidx = {}
        fidx = {}
        # b0 = tiles 0..3, b1 = tiles 4..6, b2 = tile 7
        cidx[0] = bt.tile([P, 4], I16, tag="cidx0", name="cidx0")
        cidx[1] = bt.tile([P, 3], I16, tag="cidx1", name="cidx1")
        cidx[2] = bt.tile([P, 1], I16, tag="cidx2", name="cidx2")
        reds = []
        for t in range(4):
            reds.append(reduce_pick(t, t, cidx[0]))
        reds.append(reduce_pick(4, 0, cidx[1]))
        idxw_c0 = wrap(cidx[0], 4, "scr_c0", "idxw_c0", nc.gpsimd)
        wch0 = gather_c(0, 4, idxw_c0)
        reds.append(reduce_pick(5, 1, cidx[1]))
        fidx[0] = bt.tile([P, 4], I16, tag="fidx0", name="fidx0")
        for j in range(4):
            within(j, j, wch0, fidx[0])
        idxw_w0 = wrap(fidx[0], 4, "scr_w0", "idxw_w0", nc.gpsimd)
        wout0 = gather_w(4, idxw_w0)
        store(0, 4, wout0)
        reduce_pick(6, 2, cidx[1])
        idxw_c1 = wrap(cidx[1], 3, "scr_c1", "idxw_c1", nc.scalar)
        wch1 = gather_c(4, 3, idxw_c1)
        reduce_pick(7, 0, cidx[2])
        idxw_c2 = wrap(cidx[2], 1, "scr_c2", "idxw_c2", nc.scalar)
        wch2 = gather_c(7, 1, idxw_c2)
        fidx[1] = bt.tile([P, 3], I16, tag="fidx1", name="fidx1")
        for j in range(3):
            within(4 + j, j, wch1, fidx[1])
        idxw_w1 = wrap(fidx[1], 3, "scr_w1", "idxw_w1", nc.scalar)
        wout1 = gather_w(3, idxw_w1)
        store(4, 3, wout1)
        fidx[2] = bt.tile([P, 1], I16, tag="fidx2", name="fidx2")
        within(7, 0, wch2, fidx[2])
        idxw_w2 = wrap(fidx[2], 1, "scr_w2", "idxw_w2", nc.scalar)
        wout2 = gather_w(1, idxw_w2)
        store(7, 1, wout2)


def _emit_kernel_v8(tc: tile.TileContext, y: "bass.AP", x: "bass.AP", wt: "bass.AP"):
    """Per-tile pipeline built on indirect_dma_start (natural [P,1] indices).

    v7's tail cost came from the dma_gather index-wrap machinery: every batch
    paid a DRAM scratch roundtrip x2 (25us completion latencies under load
    traffic) plus 0xf0 ucode dispatches, and batch tails were gated on the
    LAST tile of the batch, so ~85us of serial tail ran after the final load.

    v8 drops dma_gather entirely. Per 128-row tile:
      reduce_max -> chunk maxes m[P,32] -> max/max_index pick the winning
      chunk -> indirect_dma_start re-gathers each row's winning 1KB chunk
      (offsets straight from SBUF [P,1] i32 - no wrap, no scratch) ->
      max_index within the chunk -> indirect_dma_start gathers W.T rows ->
      store. Tiles pipeline independently; only the last tile's ~18us chain
      trails the final load, and tile 7 is loaded in 3 column pieces so its
      final reduce is 2.2us instead of 8.7us.

    Engine streams: Sync = x loads only; Scalar = y stores; GpSimd = the two
    indirect gathers per tile; Vector = reduce/pick/find with finds staggered
    one tile behind reduces so the in-order stream never parks on a gather.
    """
    nc = tc.nc
    with (
        tc.tile_pool(name="xp", bufs=5) as xp,
        tc.tile_pool(name="mp", bufs=2) as mp,
        tc.tile_pool(name="sm", bufs=3) as sm,
        tc.tile_pool(name="pk", bufs=N_TILES) as pk,
        tc.tile_pool(name="ii", bufs=3) as ii,
        tc.tile_pool(name="wc", bufs=3) as wc,
        tc.tile_pool(name="fi", bufs=3) as fi,
        tc.tile_pool(name="wo", bufs=3) as wo,
        tc.tile_pool(name="keep", bufs=1) as keep,
    ):
        iota32 = keep.tile([P, 1], F32)  # p*NCH per partition
        nc.gpsimd.iota(
            iota32[:],
            pattern=[[0, 1]],
            base=0,
            channel_multiplier=NCH,
            allow_small_or_imprecise_dtypes=True,
        )

        # x viewed as a flat chunk table for the winning-chunk re-gather.
        x_chunks = x.rearrange("r (c e) -> (r c) e", e=CH)

        # Queue every HBM load up front on the Sync HWDGE ring. Tile 7 goes
        # in three column pieces so the tail's final reduce is small.
        T_LAST = N_TILES - 1
        xts = []
        for t in range(T_LAST):
            xt = xp.tile([P, QUANT_DIM], F32, tag="xt")
            nc.sync.dma_start(xt[:], x[t * P : (t + 1) * P, :])
            xts.append(xt)
        xt7 = xp.tile([P, QUANT_DIM], F32, tag="xt")
        xts.append(xt7)
        PIECES = [(0, 4096), (4096, 6144), (6144, 8192)]
        for c0, c1 in PIECES:
            nc.sync.dma_start(
                xt7[:, c0:c1], x[T_LAST * P : (T_LAST + 1) * P, c0:c1]
            )

        mx8s = [None] * N_TILES
        crfs = [None] * N_TILES
        cidxs = [None] * N_TILES
        wchs = [None] * N_TILES
        fidxs = [None] * N_TILES
        wouts = [None] * N_TILES
        m7 = None

        def reduce_t(t):
            m = mp.tile([P, NCH], F32, tag="m")
            nc.vector.reduce_max(
                m[:],
                xts[t].rearrange("p (c e) -> p c e", e=CH),
                axis=mybir.AxisListType.X,
            )
            return m

        def pick_t(t, m):
            """winning chunk + global chunk id for the re-gather"""
            mx8 = pk.tile([P, 8], F32, tag="mx8")
            mx8s[t] = mx8
            nc.vector.max(mx8[:], m[:])
            ci8 = sm.tile([P, 8], U32, tag="ci8")
            nc.vector.max_index(ci8[:], mx8[:], m[:])
            crf = pk.tile([P, 1], F32, tag="crf")
            crfs[t] = crf
            nc.vector.tensor_copy(crf[:], ci8[:, 0:1])  # u32 -> f32
            gci = sm.tile([P, 1], F32, tag="gci")
            # chunk id in x_chunks: (t*128 + p)*NCH + cr
            nc.vector.tensor_scalar(
                gci[:],
                crf[:],
                float(t * P * NCH),
                iota32[:],
                op0=mybir.AluOpType.add,
                op1=mybir.AluOpType.add,
            )
            cidx = ii.tile([P, 1], I32, tag="cidx")
            cidxs[t] = cidx
            nc.vector.tensor_copy(cidx[:], gci[:])  # f32 -> i32

        def chunk_gather(t):
            wch = wc.tile([P, CH], F32, tag="wch")
            wchs[t] = wch
            nc.gpsimd.indirect_dma_start(
                out=wch[:],
                out_offset=None,
                in_=x_chunks,
                in_offset=bass.IndirectOffsetOnAxis(ap=cidxs[t][:, :1], axis=0),
            )

        def find_t(t):
            """offset within the winning chunk -> final row argmax"""
            wix8 = sm.tile([P, 8], U32, tag="wix8")
            nc.vector.max_index(wix8[:], mx8s[t][:], wchs[t][:])
            wif = sm.tile([P, 1], F32, tag="wif")
            nc.vector.tensor_copy(wif[:], wix8[:, 0:1])  # u32 -> f32
            fin = sm.tile([P, 1], F32, tag="fin")
            nc.vector.tensor_scalar(
                fin[:],
                crfs[t][:],
                float(CH),
                wif[:],
                op0=mybir.AluOpType.mult,
                op1=mybir.AluOpType.add,
            )
            fidx = fi.tile([P, 1], I32, tag="fidx")
            fidxs[t] = fidx
            nc.vector.tensor_copy(fidx[:], fin[:])  # f32 -> i32

        def w_gather(t):
            wout = wo.tile([P, OUT_DIM], F32, tag="wout")
            wouts[t] = wout
            nc.gpsimd.indirect_dma_start(
                out=wout[:],
                out_offset=None,
                in_=wt[:],
                in_offset=bass.IndirectOffsetOnAxis(ap=fidxs[t][:, :1], axis=0),
            )

        def store_t(t):
            nc.scalar.dma_start(y[t * P : (t + 1) * P, :], wouts[t][:])

        # Global emission order sets per-engine stream priorities:
        #   Vector: R0 R1 F0 R2 F1 ... R6 F5 R7a R7b F6 R7c pick7 F7
        #   GpSimd: CG0 CG1 WG0 CG2 WG1 ... CG6 WG5 WG6 CG7 WG7
        pick_t(0, reduce_t(0))
        chunk_gather(0)
        for t in range(1, T_LAST):
            pick_t(t, reduce_t(t))
            chunk_gather(t)
            find_t(t - 1)
            w_gather(t - 1)
            store_t(t - 1)
        # tile 7: reduce arrives in three pieces
        m7 = mp.tile([P, NCH], F32, tag="m")
        bounds = [0] + [c1 // CH for _, c1 in PIECES]
        nc.vector.reduce_max(
            m7[:, bounds[0] : bounds[1]],
            xt7[:, : PIECES[0][1]].rearrange("p (c e) -> p c e", e=CH),
            axis=mybir.AxisListType.X,
        )
        nc.vector.reduce_max(
            m7[:, bounds[1] : bounds[2]],
            xt7[:, PIECES[1][0] : PIECES[1][1]].rearrange(
                "p (c e) -> p c e", e=CH
            ),
            axis=mybir.AxisListType.X,
        )
        find_t(T_LAST - 1)
        nc.vector.reduce_max(
            m7[:, bounds[2] : bounds[3]],
            xt7[:, PIECES[2][0] : PIECES[2][1]].rearrange(
                "p (c e) -> p c e", e=CH
            ),
            axis=mybir.AxisListType.X,
        )
        pick_t(T_LAST, m7)
        w_gather(T_LAST - 1)
        store_t(T_LAST - 1)
        chunk_gather(T_LAST)
        find_t(T_LAST)
        w_gather(T_LAST)
        store_t(T_LAST)


def _emit_kernel_v9(tc: tile.TileContext, y: "bass.AP", x: "bass.AP", wt: "bass.AP"):
    """v8 + three fixes from the v8 trace:

    - find(t) is pinned behind reduce(t+1) with add_dep_helper: the Tile
      scheduler's cost model thinks the chunk gather is fast and otherwise
      hoists find(t) ahead of the next reduce, parking the in-order Vector
      stream ~11us per tile on the gather's DMA-completion semaphore.
    - The index arithmetic collapses to one int tensor_scalar per stage
      (cidx = ci + t*4096 + p*32; fidx = (ci << 8) + wi), replacing the
      u32->f32->i32 CAST chains (~1.5us/tile of Vector stream time).
    - Loads go as 2MB column halves (reduce per half), so the first reduce
      starts ~7us earlier, SBUF pool slots recycle at half-tile granularity,
      and the slot wait for load t+5 binds ~like the load pace instead of
      behind it. y stores ride the Sync ring after all load triggers.
    """
    nc = tc.nc
    H = QUANT_DIM // 2  # 4096
    with (
        tc.tile_pool(name="xp", bufs=5) as xp,
        tc.tile_pool(name="mp", bufs=2) as mp,
        tc.tile_pool(name="sm", bufs=3) as sm,
        tc.tile_pool(name="pk", bufs=3) as pk,
        tc.tile_pool(name="ii", bufs=3) as ii,
        tc.tile_pool(name="wc", bufs=3) as wc,
        tc.tile_pool(name="fi", bufs=3) as fi,
        tc.tile_pool(name="wo", bufs=3) as wo,
        tc.tile_pool(name="keep", bufs=1) as keep,
    ):
        iota_i = keep.tile([P, 1], I32)  # p*NCH per partition
        nc.gpsimd.iota(iota_i[:], pattern=[[0, 1]], base=0, channel_multiplier=NCH)

        x_chunks = x.rearrange("r (c e) -> (r c) e", e=CH)

        # Loads: two 2MB column halves per tile; tile 7's second half goes in
        # two 1MB quarters so the tail's final reduce is 2.2us.
        T_LAST = N_TILES - 1
        xts = []
        for t in range(T_LAST):
            xt = xp.tile([P, QUANT_DIM], F32, tag="xt")
            nc.sync.dma_start(xt[:, 0:H], x[t * P : (t + 1) * P, 0:H])
            nc.sync.dma_start(xt[:, H:], x[t * P : (t + 1) * P, H:])
            xts.append(xt)
        xt7 = xp.tile([P, QUANT_DIM], F32, tag="xt")
        xts.append(xt7)
        PIECES7 = [(0, H), (H, H + H // 2), (H + H // 2, QUANT_DIM)]
        for c0, c1 in PIECES7:
            nc.sync.dma_start(
                xt7[:, c0:c1], x[T_LAST * P : (T_LAST + 1) * P, c0:c1]
            )

        mx8s = [None] * N_TILES
        ci8s = [None] * N_TILES
        cidxs = [None] * N_TILES
        wchs = [None] * N_TILES
        fidxs = [None] * N_TILES
        wouts = [None] * N_TILES

        def reduce_cols(t, m, c0, c1):
            return nc.vector.reduce_max(
                m[:, c0 // CH : c1 // CH],
                xts[t][:, c0:c1].rearrange("p (c e) -> p c e", e=CH),
                axis=mybir.AxisListType.X,
            )

        def pick_t(t, m):
            mx8 = pk.tile([P, 8], F32, tag="mx8")
            mx8s[t] = mx8
            nc.vector.max(mx8[:], m[:])
            ci8 = pk.tile([P, 8], U32, tag="ci8")
            ci8s[t] = ci8
            nc.vector.max_index(ci8[:], mx8[:], m[:])
            cidx = ii.tile([P, 1], I32, tag="cidx")
            cidxs[t] = cidx
            # chunk id in x_chunks: (t*128 + p)*NCH + ci. The three fields
            # occupy disjoint bit ranges (ci<32, p*32, t*4096), so OR == add
            # and keeps the whole computation in int32.
            nc.vector.tensor_scalar(
                cidx[:],
                ci8.bitcast(I32)[:, 0:1],
                int(t * P * NCH),
                iota_i[:],
                op0=mybir.AluOpType.bitwise_or,
                op1=mybir.AluOpType.bitwise_or,
            )

        def chunk_gather(t):
            wch = wc.tile([P, CH], F32, tag="wch")
            wchs[t] = wch
            nc.gpsimd.indirect_dma_start(
                out=wch[:],
                out_offset=None,
                in_=x_chunks,
                in_offset=bass.IndirectOffsetOnAxis(ap=cidxs[t][:, :1], axis=0),
            )

        def find_t(t, after=None):
            wix8 = sm.tile([P, 8], U32, tag="wix8")
            wix = nc.vector.max_index(wix8[:], mx8s[t][:], wchs[t][:])
            if after is not None:
                # Hold this off the Vector stream until `after` has issued;
                # the scheduler would otherwise park the stream here waiting
                # for the chunk gather.
                tile.add_dep_helper(
                    wix.ins, after.ins, sync=False, reason="stagger find"
                )
            fidx = fi.tile([P, 1], I32, tag="fidx")
            fidxs[t] = fidx
            # final row argmax: (ci << 8) + wi
            nc.vector.tensor_scalar(
                fidx[:],
                ci8s[t].bitcast(I32)[:, 0:1],
                8,
                wix8.bitcast(I32)[:, 0:1],
                op0=mybir.AluOpType.logical_shift_left,
                op1=mybir.AluOpType.bitwise_or,
            )

        def w_gather(t):
            wout = wo.tile([P, OUT_DIM], F32, tag="wout")
            wouts[t] = wout
            nc.gpsimd.indirect_dma_start(
                out=wout[:],
                out_offset=None,
                in_=wt[:],
                in_offset=bass.IndirectOffsetOnAxis(ap=fidxs[t][:, :1], axis=0),
            )

        def store_t(t):
            nc.sync.dma_start(y[t * P : (t + 1) * P, :], wouts[t][:])

        # Vector: r0a r0b pick0 | r1a r1b pick1 F0 | ... | r6a r6b pick6 F5 |
        #         r7a r7b F6 r7c pick7 F7
        # GpSimd: CG0 CG1 WG0 CG2 WG1 ... CG6 WG5 WG6 CG7 WG7
        ms = [None] * N_TILES
        ms[0] = mp.tile([P, NCH], F32, tag="m", name="m0")
        reduce_cols(0, ms[0], 0, H)
        r_last = reduce_cols(0, ms[0], H, QUANT_DIM)
        pick_t(0, ms[0])
        chunk_gather(0)
        for t in range(1, T_LAST):
            ms[t] = mp.tile([P, NCH], F32, tag="m", name=f"m{t}")
            reduce_cols(t, ms[t], 0, H)
            r_last = reduce_cols(t, ms[t], H, QUANT_DIM)
            pick_t(t, ms[t])
            chunk_gather(t)
            find_t(t - 1, after=r_last)
            w_gather(t - 1)
            store_t(t - 1)
        m7 = mp.tile([P, NCH], F32, tag="m")
        ms[T_LAST] = m7
        reduce_cols(T_LAST, m7, *PIECES7[0])
        r7b = reduce_cols(T_LAST, m7, *PIECES7[1])
        find_t(T_LAST - 1, after=r7b)
        reduce_cols(T_LAST, m7, *PIECES7[2])
        pick_t(T_LAST, m7)
        w_gather(T_LAST - 1)
        store_t(T_LAST - 1)
        chunk_gather(T_LAST)
        find_t(T_LAST)
        w_gather(T_LAST)
        store_t(T_LAST)


_CACHE: dict[str, object] = {}


def _build():
    if "nc" in _CACHE:
        return _CACHE["nc"]
    nc = bacc.Bacc(
        "TRN2",
        target_bir_lowering=False,
        debug=False,
        enable_asserts=True,
        num_swdge_queues=1,
    )
    x = nc.dram_tensor("x", [ROWS, QUANT_DIM], F32, kind="ExternalInput").ap()
    wt = nc.dram_tensor("wt", [QUANT_DIM, OUT_DIM], F32, kind="ExternalInput").ap()
    y = nc.dram_tensor("y", [ROWS, OUT_DIM], F32, kind="ExternalOutput").ap()
    emit = {
        1: _emit_kernel,
        2: _emit_kernel_v2,
        3: _emit_kernel_v3,
        4: _emit_kernel_v4,
        5: _emit_kernel_v5,
        6: _emit_kernel_v6,
        7: _emit_kernel_v7,
        8: _emit_kernel_v8,
        9: _emit_kernel_v9,
    }[VERSION]
    with tile.TileContext(nc) as tc:
        emit(tc, y, x, wt)
    nc.compile()
    _CACHE["nc"] = nc
    return nc


def kernel(x: np.ndarray, W: np.ndarray, **_unused) -> np.ndarray:
    assert x.shape == (N_TOKENS, QUANT_DIM) and W.shape == (OUT_DIM, QUANT_DIM)
    nc = _build()
    x = np.ascontiguousarray(x, dtype=np.float32)
    wt = np.ascontiguousarray(W.T.astype(np.float32, copy=False))
    in_maps = [
        {"x": x[i * ROWS : (i + 1) * ROWS], "wt": wt} for i in range(N_CORES)
    ]
    res = bass_utils.run_bass_kernel_spmd(nc, in_maps, core_ids=list(range(N_CORES)))
    return np.concatenate([res.results[i]["y"] for i in range(N_CORES)], axis=0)

